# revision 29
# baseline (speedup 1.0000x reference)
"""Trainium2 Bass kernel for MiniJMamba (12-layer SSM+attention hybrid).

Sharding: 8 cores = batch(4) x seq-half(2). Each core processes a
1152-token window (1024 local + 64-token halo each side, zero-padded at
sequence ends). The 10 depthwise convs each consume one token of halo
validity (10 <= 64). At the two attention layers, cores exchange K/V
halves with their pair core via AllGather over groups [2b, 2b+1].

Host-side prep folds every LayerNorm affine into the following matmul
weights, so on-device LN is pure normalization. Activations use two
layouts: token-major [128 tok, ...] for LN and feature-major [feat, tok]
as matmul lhsT; PE transposes bridge them. Matmul operands are bf16
(PSUM accumulation fp32); the residual stream and LN stats stay fp32.
"""

import os
import numpy as np

B, S, F, D, H, V = 4, 2048, 512, 512, 8, 1024
HD, FFD = 64, 2048
NSSM, NATTN = 10, 2
ATTN_POS = (4, 8)
EPS = 1e-5
HALO = 64
T = 1024 + 2 * HALO            # 1152 tokens per core window
NT = T // 128                  # 9 token tiles
KC = D // 128                  # 4 feature chunks
PANELS = [(0, 512), (512, 512), (1024, T - 1024)]  # free-dim panels of T
# attention query window: local tokens + 6-token halo each side (enough for
# the convolutions remaining after each attention layer)
QPANELS = [(58, 512), (570, 512), (1082, 12)]
NKV = S // 128                 # 16 key chunks (full sequence)

_CACHE = {}


# ----------------------------------------------------------------- build

def _build_nc(dt_mm_name="bfloat16"):
    import concourse.bass as bass
    import concourse.tile as tile
    from concourse import bacc, mybir
    from concourse.masks import make_identity
    import contextlib

    F32 = mybir.dt.float32
    DTM = getattr(mybir.dt, dt_mm_name)
    AF = mybir.ActivationFunctionType
    ALU = mybir.AluOpType
    ts = bass.ts

    nc = bacc.Bacc(None, target_bir_lowering=False, num_devices=8)

    def din(name, shape, dt=DTM):
        return nc.dram_tensor(name, shape, dt, kind="ExternalInput")

    # ---- inputs
    framesT = din("framesT", [D, T])
    smask = din("smask", [1, T])
    cosB = din("cosB", [128, T])
    sinB = din("sinB", [128, T])
    w_in = din("w_in", [F, D]); b_in = din("b_in", [1, D])
    w_si = din("w_si", [NSSM, D, 2 * D]); b_si = din("b_si", [NSSM, 128, 8], F32)
    cw = din("cw", [NSSM, 128, KC, 3], F32); cb = din("cb", [NSSM, 128, KC], F32)
    w_so = din("w_so", [NSSM, D, D]); b_so = din("b_so", [NSSM, 1, D])
    w_q = din("w_q", [NATTN, D, D]); b_q = din("b_q", [NATTN, 128, KC], F32)
    w_k = din("w_k", [NATTN, D, D]); b_k = din("b_k", [NATTN, 128, KC], F32)
    w_qr = din("w_qr", [NATTN, D, D]); b_qr = din("b_qr", [NATTN, 128, KC], F32)
    w_kr = din("w_kr", [NATTN, D, D]); b_kr = din("b_kr", [NATTN, 128, KC], F32)
    w_v = din("w_v", [NATTN, D, D]); b_v = din("b_v", [NATTN, 1, D])
    w_ao = din("w_ao", [NATTN, D, D]); b_ao = din("b_ao", [NATTN, 1, D])
    w_f1 = din("w_f1", [NATTN, D, FFD]); b_f1 = din("b_f1", [NATTN, 128, FFD // 128], F32)
    w_f2 = din("w_f2", [NATTN, FFD, D]); b_f2 = din("b_f2", [NATTN, 1, D])
    w_fr = din("w_fr", [D, F]); b_fr = din("b_fr", [1, F])
    w_sy = din("w_sy", [D, V]); b_sy = din("b_sy", [1, V])

    out_frame = nc.dram_tensor("out_frame", [T, F], F32, kind="ExternalOutput")
    out_sym = nc.dram_tensor("out_sym", [T, V], F32, kind="ExternalOutput")

    RG = [[0, 1], [2, 3], [4, 5], [6, 7]]

    with tile.TileContext(nc) as tc:
        ctx = contextlib.ExitStack()
        with ctx:
            persist = ctx.enter_context(tc.tile_pool(name="persist", bufs=1))
            wpool = ctx.enter_context(tc.tile_pool(name="wpool", bufs=2))
            act = ctx.enter_context(tc.tile_pool(name="act", bufs=1))
            big = ctx.enter_context(tc.tile_pool(name="big", bufs=1))
            sm2 = ctx.enter_context(tc.tile_pool(name="sm2", bufs=2))
            sm1 = ctx.enter_context(tc.tile_pool(name="sm1", bufs=1))
            wpool1 = ctx.enter_context(tc.tile_pool(name="wpool1", bufs=1))
            sm3 = ctx.enter_context(tc.tile_pool(name="sm3", bufs=3))
            ps_mm = ctx.enter_context(tc.tile_pool(name="ps_mm", bufs=3, space="PSUM"))
            ps_av = ctx.enter_context(tc.tile_pool(name="ps_av", bufs=3, space="PSUM"))
            ps_tr = ctx.enter_context(tc.tile_pool(name="ps_tr", bufs=2, space="PSUM"))
            dram = ctx.enter_context(tc.tile_pool(name="dram", bufs=2, space="DRAM"))

            ident = persist.tile([128, 128], DTM)
            make_identity(nc, ident)
            ones_f32 = persist.tile([1, 64], F32)
            nc.vector.memset(ones_f32[:], 1.0)
            epst = persist.tile([128, 1], F32)
            nc.vector.memset(epst[:], EPS)
            zerob = persist.tile([128, 1], F32)
            nc.vector.memset(zerob[:], 0.0)

            smask_bc = persist.tile([128, T], DTM)
            nc.sync.dma_start(smask_bc[:], smask[:].to_broadcast((128, T)))
            cosB_sb = persist.tile([128, T], DTM)
            nc.sync.dma_start(cosB_sb[:], cosB[:])
            sinB_sb = persist.tile([128, T], DTM)
            nc.sync.dma_start(sinB_sb[:], sinB[:])

            x = persist.tile([128, NT, D], F32)     # residual stream, token-major

            # ---------- helpers ----------
            def load_w(w2d, kchunks, n_out, tag):
                pool_ = wpool1 if tag == "w_d_d2" else wpool
                t_ = pool_.tile([128, kchunks, n_out], DTM, tag=tag)
                nc.sync.dma_start(t_[:], w2d.rearrange("(kc p) n -> p kc n", p=128))
                return t_

            def load_bias_fm(b2d, ncols, tag):
                t_ = wpool.tile([128, ncols], F32, tag=tag)
                nc.sync.dma_start(t_[:], b2d)
                return t_

            def load_bias_bc(b2d, n, tag):
                t_ = wpool.tile([128, n], DTM, tag=tag)
                nc.sync.dma_start(t_[:], b2d.to_broadcast((128, n)))
                return t_

            def mm_fm(out_fm, actT, w_sb, bias_fm, nchunks, epi_func, off=0):
                """out_fm[:, off+n, :] = epi(w-cols.T @ actT-chunks + bias)."""
                for n in range(nchunks):
                    for (qo, qw) in PANELS:
                        ps = ps_mm.tile([128, 512], F32, tag="ps_mm")
                        for kc_ in range(KC):
                            nc.tensor.matmul(
                                ps[:, :qw], lhsT=w_sb[:, kc_, ts(off + n, 128)],
                                rhs=actT[:, kc_, qo:qo + qw],
                                start=(kc_ == 0), stop=(kc_ == KC - 1))
                        nc.scalar.activation(
                            out=out_fm[:, off + n, qo:qo + qw], in_=ps[:, :qw],
                            func=epi_func, bias=bias_fm[:, off + n:off + n + 1])

            def mm_tm(actT, kchunks, w_sb, n_out, consumer):
                """token-major out: per t-tile psum [128, n<=512] -> consumer
                (bias is applied by the consumer, off the PE)."""
                nh = (n_out + 511) // 512
                for t_ in range(NT):
                    for nh_i in range(nh):
                        n0 = nh_i * 512
                        nw = min(512, n_out - n0)
                        ps = ps_mm.tile([128, 512], F32, tag="ps_mm")
                        for kc_ in range(kchunks):
                            nc.tensor.matmul(
                                ps[:, :nw], lhsT=actT[:, kc_, ts(t_, 128)],
                                rhs=w_sb[:, kc_, n0:n0 + nw],
                                start=(kc_ == 0), stop=(kc_ == kchunks - 1))
                        consumer(t_, ps[:, :nw], n0, nw)

            def layernorm_T(xt):
                """Pure-normalize x (token-major fp32) -> transposed [128, KC, T] bf16.

                Fully per-tile pipeline: tile t's transposes (PE) overlap tile
                t+1's stats (DVE) instead of a batched-stats barrier."""
                xhatT = act.tile([128, KC, T], DTM, tag="xhatT", name="xhatT")
                for t_ in range(NT):
                    st6 = sm3.tile([128, 6], F32, tag="ln_st6", name="ln_st6")
                    mv = sm3.tile([128, 2], F32, tag="ln_mv", name="ln_mv")
                    rstd = sm3.tile([128, 1], F32, tag="ln_rstd", name="ln_rstd")
                    nc.vector.bn_stats(st6[:], xt[:, t_, :])
                    nc.vector.bn_aggr(mv[:], st6[:])
                    nc.scalar.activation(out=rstd[:], in_=mv[:, 1:2], func=AF.Sqrt,
                                         bias=epst[:])
                    nc.vector.reciprocal(rstd[:], rstd[:])
                    xh = sm2.tile([128, D], DTM, tag="ln_xh", name="ln_xh")
                    nc.vector.tensor_scalar(
                        out=xh[:], in0=xt[:, t_, :],
                        scalar1=mv[:, 0:1], scalar2=rstd[:, 0:1],
                        op0=ALU.subtract, op1=ALU.mult)
                    pt = ps_tr.tile([128, 512], DTM, tag="ps_tr", name="pt")
                    for kc_ in range(KC):
                        nc.tensor.transpose(pt[:, ts(kc_, 128)],
                                            xh[:, ts(kc_, 128)], ident[:])
                    nc.vector.tensor_copy(
                        xhatT[:, :, ts(t_, 128)],
                        pt[:].rearrange("p (c q) -> p c q", q=128))
                return xhatT

            def rope_combine(qk, qkr):
                """qk <- qk*cosB + qkr*sinB (rotation term computed via folded weights)."""
                for c in range(KC):
                    tmp = sm1.tile([128, T], F32, tag="tmp_a", name="rope_tmp")
                    sn = sm1.tile([128, T], F32, tag="tmp_b", name="rope_s")
                    nc.vector.tensor_mul(tmp[:], qk[:, c, :], cosB_sb[:])
                    nc.gpsimd.tensor_mul(sn[:], qkr[:, c, :], sinB_sb[:])
                    nc.vector.tensor_add(qk[:, c, :], tmp[:], sn[:])

            # ---------- input projection ----------
            framesT_sb = act.tile([128, KC, T], DTM, tag="xhatT", name="framesT_sb")
            nc.sync.dma_start(framesT_sb[:],
                              framesT[:].rearrange("(kc p) t -> p kc t", p=128))
            w_in_sb = load_w(w_in[:], KC, D, "w_d_d")
            b_in_sb = load_bias_bc(b_in[:], D, "b_row_d")

            def into_x(t_, ps, n0, nw):
                nc.vector.tensor_add(x[:, t_, n0:n0 + nw], ps, b_in_sb[:, n0:n0 + nw])
            mm_tm(framesT_sb, KC, w_in_sb, D, into_x)

            # ---------- layers ----------
            si = ai = 0
            for layer in range(NSSM + NATTN):
                if layer in ATTN_POS:
                    a = ai; ai += 1
                    # === attention block ===
                    xhatT = layernorm_T(x)

                    # K projection first so the exchange starts early
                    w_k_sb = load_w(w_k[a], KC, D, "w_d_d")
                    b_k_sb = load_bias_fm(b_k[a], KC, "b_fm_4")
                    k_fm = act.tile([128, KC, T], DTM, tag="mix2", name="k_fm")
                    mm_fm(k_fm, xhatT, w_k_sb, b_k_sb, KC, AF.Identity)
                    w_kr_sb = load_w(w_kr[a], KC, D, "w_d_d")
                    b_kr_sb = load_bias_fm(b_kr[a], KC, "b_fm_4")
                    kr_fm = act.tile([128, KC, T], DTM, tag="rot", name="kr_fm")
                    mm_fm(kr_fm, xhatT, w_kr_sb, b_kr_sb, KC, AF.Identity)
                    rope_combine(k_fm, kr_fm)
                    k_bin = dram.tile([D, 1024], DTM, tag="k_bin", name="k_bin")
                    nc.sync.dma_start(
                        k_bin[:].rearrange("(c p) t -> p c t", p=128),
                        k_fm[:, :, HALO:HALO + 1024])
                    k_bout = dram.tile([2 * D, 1024], DTM, tag="k_bout", name="k_bout")
                    nc.gpsimd.collective_compute(
                        "AllGather", ALU.bypass, replica_groups=RG,
                        ins=[k_bin.opt()], outs=[k_bout.opt()])
                    k_full = big.tile([128, KC, S], DTM, tag="k_full", name="k_full")
                    nc.sync.dma_start(
                        k_full[:, :, 0:1024],
                        k_bout[0:D].rearrange("(c p) t -> p c t", p=128))
                    nc.sync.dma_start(
                        k_full[:, :, 1024:2048],
                        k_bout[D:2 * D].rearrange("(c p) t -> p c t", p=128))

                    # V projection (token-major) -> exchange
                    w_v_sb = load_w(w_v[a], KC, D, "w_d_d")
                    b_v_sb = load_bias_bc(b_v[a], D, "b_row_d")
                    v_loc = act.tile([128, NT, D], DTM, tag="mix1", name="v_loc")

                    def into_v(t_, ps, n0, nw):
                        nc.vector.tensor_add(v_loc[:, t_, n0:n0 + nw], ps,
                                             b_v_sb[:, n0:n0 + nw])
                    mm_tm(xhatT, KC, w_v_sb, D, into_v)
                    v_bin = dram.tile([1024, D], DTM, tag="v_bin", name="v_bin")
                    nc.sync.dma_start(v_bin[0:64], v_loc[64:128, 0, :])
                    nc.sync.dma_start(
                        v_bin[64:960].rearrange("(c p) n -> p c n", p=128),
                        v_loc[:, 1:8, :])
                    nc.sync.dma_start(v_bin[960:1024], v_loc[0:64, 8, :])
                    v_bout = dram.tile([S, D], DTM, tag="v_bout", name="v_bout")
                    nc.gpsimd.collective_compute(
                        "AllGather", ALU.bypass, replica_groups=RG,
                        ins=[v_bin.opt()], outs=[v_bout.opt()])
                    v_tm = big.tile([128, NKV, H, HD + 1], DTM, tag="v_tm", name="v_tm")
                    for h_ in range(H):
                        nc.sync.dma_start(
                            v_tm[:, :, h_, 0:HD],
                            v_bout[:, h_ * HD:(h_ + 1) * HD].rearrange(
                                "(kc p) d -> p kc d", p=128))
                    nc.vector.memset(v_tm[:, :, :, HD:HD + 1], 1.0)

                    # Q projection (overlaps the collectives)
                    w_q_sb = load_w(w_q[a], KC, D, "w_d_d")
                    b_q_sb = load_bias_fm(b_q[a], KC, "b_fm_4")
                    q_fm = act.tile([128, KC, T], DTM, tag="q_fm", name="q_fm")
                    mm_fm(q_fm, xhatT, w_q_sb, b_q_sb, KC, AF.Identity)
                    w_qr_sb = load_w(w_qr[a], KC, D, "w_d_d")
                    b_qr_sb = load_bias_fm(b_qr[a], KC, "b_fm_4")
                    qr_fm = act.tile([128, KC, T], DTM, tag="rot", name="qr_fm")
                    mm_fm(qr_fm, xhatT, w_qr_sb, b_qr_sb, KC, AF.Identity)
                    rope_combine(q_fm, qr_fm)

                    # attention core: q restricted to the columns whose
                    # outputs are still needed (local + remaining conv halo)
                    ctx_fm = act.tile([128, KC, T], DTM, tag="mix1", name="ctx_fm")
                    nc.vector.memset(ctx_fm[:], 0.0)
                    for h in range(H):
                        r0 = (h % 2) * 64
                        ch = h // 2
                        av_tiles = [ps_av.tile([65, 512], F32, tag="ps_av", name="av")
                                    for _ in PANELS]
                        for c in range(NKV):
                            ex = sm3.tile([128, T], DTM, tag="ebuf", name="ex")
                            for j, (qo, qw) in enumerate(QPANELS):
                                ps = ps_mm.tile([128, 512], F32, tag="ps_mm", name="sc")
                                nc.tensor.matmul(
                                    ps[:, :qw],
                                    lhsT=k_full[r0:r0 + 64, ch, ts(c, 128)],
                                    rhs=q_fm[r0:r0 + 64, ch, qo:qo + qw],
                                    start=True, stop=True)
                                nc.scalar.activation(
                                    out=ex[:, qo:qo + qw], in_=ps[:, :qw],
                                    func=AF.Exp, scale=0.125)
                            for j, (qo, qw) in enumerate(QPANELS):
                                nc.tensor.matmul(
                                    av_tiles[j][:, :qw],
                                    lhsT=v_tm[:, c, h, :],
                                    rhs=ex[:, qo:qo + qw],
                                    start=(c == 0), stop=(c == NKV - 1))
                        inv = sm1.tile([1, T], F32, tag="attn_inv", name="inv")
                        invb = sm1.tile([64, T], F32, tag="attn_invb", name="invb")
                        for j, (qo, qw) in enumerate(QPANELS):
                            nc.vector.reciprocal(inv[:, qo:qo + qw],
                                                 av_tiles[j][64:65, :qw])
                            ib_ps = ps_tr.tile([64, 512], F32, tag="ps_tr", name="ib")
                            nc.tensor.matmul(ib_ps[:, :qw],
                                             lhsT=ones_f32[:],
                                             rhs=inv[:, qo:qo + qw],
                                             start=True, stop=True)
                            nc.vector.tensor_copy(invb[:, qo:qo + qw], ib_ps[:, :qw])
                        if r0 == 0:
                            for j, (qo, qw) in enumerate(QPANELS):
                                nc.vector.tensor_mul(
                                    ctx_fm[0:64, ch, qo:qo + qw],
                                    av_tiles[j][0:64, :qw], invb[:, qo:qo + qw])
                        else:
                            stage = sm2.tile([64, T], DTM, tag="ctx_stage", name="cst")
                            for j, (qo, qw) in enumerate(QPANELS):
                                nc.vector.tensor_mul(
                                    stage[:, qo:qo + qw],
                                    av_tiles[j][0:64, :qw], invb[:, qo:qo + qw])
                            nc.sync.dma_start(ctx_fm[64:128, ch, :], stage[:])

                    # output projection + residual
                    w_ao_sb = load_w(w_ao[a], KC, D, "w_d_d")
                    b_ao_sb = load_bias_bc(b_ao[a], D, "b_row_d")

                    def into_res(t_, ps, n0, nw):
                        nc.vector.tensor_add(x[:, t_, n0:n0 + nw],
                                             x[:, t_, n0:n0 + nw], ps)
                        nc.gpsimd.tensor_add(x[:, t_, n0:n0 + nw],
                                             x[:, t_, n0:n0 + nw],
                                             b_ao_sb[:, n0:n0 + nw])
                    mm_tm(ctx_fm, KC, w_ao_sb, D, into_res)

                    # === FF block (quarters over the 2048 hidden dim,
                    # each quarter's f2 partial sum added straight into x) ===
                    xhatT = layernorm_T(x)
                    for hf in range(4):
                        w_f1_sb = wpool.tile([128, KC, 512], DTM, tag="w_d_d",
                                             name="w_f1_sb")
                        nc.sync.dma_start(
                            w_f1_sb[:],
                            w_f1[a][:, hf * 512:(hf + 1) * 512].rearrange(
                                "(kc p) n -> p kc n", p=128))
                        b_f1_sb = wpool.tile([128, 4], F32, tag="b_fm_4", name="b_f1_sb")
                        nc.sync.dma_start(b_f1_sb[:], b_f1[a][:, hf * 4:(hf + 1) * 4])
                        g_fm = act.tile([128, 4, T], DTM, tag="mix1", name="g_fm")
                        mm_fm(g_fm, xhatT, w_f1_sb, b_f1_sb, 4, AF.Gelu)
                        w_f2_sb = wpool.tile([128, 4, D], DTM, tag="w_d_d",
                                             name="w_f2_sb")
                        nc.sync.dma_start(
                            w_f2_sb[:],
                            w_f2[a][hf * 512:(hf + 1) * 512].rearrange(
                                "(kc p) n -> p kc n", p=128))
                        if hf == 0:
                            b_f2_bc = load_bias_bc(b_f2[a], D, "b_row_d")

                            def into_ffres(t_, ps, n0, nw):
                                nc.vector.tensor_add(x[:, t_, n0:n0 + nw],
                                                     x[:, t_, n0:n0 + nw], ps)
                                nc.gpsimd.tensor_add(x[:, t_, n0:n0 + nw],
                                                     x[:, t_, n0:n0 + nw],
                                                     b_f2_bc[:, n0:n0 + nw])
                        else:
                            def into_ffres(t_, ps, n0, nw):
                                nc.vector.tensor_add(x[:, t_, n0:n0 + nw],
                                                     x[:, t_, n0:n0 + nw], ps)
                        mm_tm(g_fm, KC, w_f2_sb, D, into_ffres)
                else:
                    i = si; si += 1
                    # === SSM block ===
                    xhatT = layernorm_T(x)
                    w_si_sb = load_w(w_si[i], KC, 2 * D, "w_d_d2")
                    b_si_sb = load_bias_fm(b_si[i], 8, "b_fm_8")
                    z_fm = act.tile([128, 8, T], DTM, tag="g_fm", name="z_fm")
                    mm_fm(z_fm, xhatT, w_si_sb, b_si_sb, 4, AF.Sigmoid, off=0)
                    mm_fm(z_fm, xhatT, w_si_sb, b_si_sb, 4, AF.Identity, off=4)
                    # gating + seq mask; cand buffer has a zero column each side
                    cand = act.tile([128, KC, T + 2], DTM, tag="mix2", name="cand")
                    nc.vector.memset(cand[:, :, 0:1], 0.0)
                    nc.vector.memset(cand[:, :, T + 1:T + 2], 0.0)
                    cw_sb = wpool.tile([128, KC, 3], F32, tag="cw", name="cw_sb")
                    nc.sync.dma_start(cw_sb[:], cw[i])
                    cb_sb = wpool.tile([128, KC], F32, tag="cb", name="cb_sb")
                    nc.sync.dma_start(cb_sb[:], cb[i])
                    conv_fm = act.tile([128, KC, T], DTM, tag="mix1", name="conv_fm")
                    # process gating+conv in T-halves so the out-projection of
                    # the first token tiles overlaps the second half's conv
                    for (g0, g1), (o0, o1) in (((0, 641), (0, 640)),
                                               ((641, T), (640, T))):
                        for c in range(KC):
                            sg = sm3.tile([128, T], DTM, tag="ebuf", name="sg")
                            nc.vector.tensor_mul(sg[:, g0:g1], z_fm[:, c, g0:g1],
                                                 z_fm[:, KC + c, g0:g1])
                            nc.gpsimd.tensor_mul(cand[:, c, 1 + g0:1 + g1],
                                                 sg[:, g0:g1], smask_bc[:, g0:g1])
                        wd = o1 - o0
                        for c in range(KC):
                            t1 = sm1.tile([128, T], F32, tag="tmp_a", name="conv_t1")
                            t2 = sm1.tile([128, T], F32, tag="tmp_b", name="conv_t2")
                            nc.vector.tensor_scalar_mul(t1[:, :wd],
                                                        cand[:, c, o0:o1],
                                                        cw_sb[:, c, 0:1])
                            nc.vector.scalar_tensor_tensor(
                                out=t2[:, :wd], in0=cand[:, c, o0 + 1:o1 + 1],
                                scalar=cw_sb[:, c, 1:2], in1=t1[:, :wd],
                                op0=ALU.mult, op1=ALU.add)
                            nc.vector.scalar_tensor_tensor(
                                out=t1[:, :wd], in0=cand[:, c, o0 + 2:o1 + 2],
                                scalar=cw_sb[:, c, 2:3], in1=t2[:, :wd],
                                op0=ALU.mult, op1=ALU.add)
                            nc.scalar.activation(
                                out=conv_fm[:, c, o0:o1], in_=t1[:, :wd],
                                func=AF.Relu, bias=cb_sb[:, c:c + 1])
                    w_so_sb = load_w(w_so[i], KC, D, "w_d_d")
                    b_so_sb = load_bias_bc(b_so[i], D, "b_row_d")

                    def into_res_s(t_, ps, n0, nw):
                        nc.vector.tensor_add(x[:, t_, n0:n0 + nw],
                                             x[:, t_, n0:n0 + nw], ps)
                        nc.gpsimd.tensor_add(x[:, t_, n0:n0 + nw],
                                             x[:, t_, n0:n0 + nw],
                                             b_so_sb[:, n0:n0 + nw])
                    mm_tm(conv_fm, KC, w_so_sb, D, into_res_s)

            # ---------- heads ----------
            hT = layernorm_T(x)
            w_fr_sb = load_w(w_fr[:], KC, F, "w_d_d")
            b_fr_sb = load_bias_bc(b_fr[:], F, "b_row_d")

            def into_frame(t_, ps, n0, nw):
                st_ = sm2.tile([128, 512], F32, tag="ln_xh", name="ost")
                nc.vector.tensor_add(st_[:, :nw], ps, b_fr_sb[:, n0:n0 + nw])
                nc.sync.dma_start(out_frame[ts(t_, 128), n0:n0 + nw], st_[:, :nw])
            mm_tm(hT, KC, w_fr_sb, F, into_frame)

            w_sy_sb = wpool1.tile([128, KC, V], DTM, tag="w_d_d2", name="w_sy_sb")
            nc.sync.dma_start(w_sy_sb[:],
                              w_sy[:].rearrange("(kc p) n -> p kc n", p=128))
            b_sy_sb = wpool1.tile([128, V], DTM, tag="b_row_2d", name="b_sy_sb")
            nc.sync.dma_start(b_sy_sb[:], b_sy[:].to_broadcast((128, V)))

            def into_sym(t_, ps, n0, nw):
                st_ = sm2.tile([128, 512], F32, tag="ln_xh", name="ost")
                nc.vector.tensor_add(st_[:, :nw], ps, b_sy_sb[:, n0:n0 + nw])
                nc.sync.dma_start(out_sym[ts(t_, 128), n0:n0 + nw], st_[:, :nw])
            mm_tm(hT, KC, w_sy_sb, V, into_sym)

    nc.finalize()
    return nc


# ------------------------------------------------------------- host prep

def _host_prep(inputs, np_mm):
    g = {k: np.asarray(v, np.float32) if np.asarray(v).dtype != np.bool_
         else np.asarray(v) for k, v in inputs.items()}

    def fm_bias(b):      # [N] -> [128, N//128]
        return np.ascontiguousarray(b.reshape(-1, 128).T).astype(np.float32)

    P = {}
    P["w_in"] = g["input_w"].astype(np_mm)
    P["b_in"] = g["input_b"][None, :].astype(np_mm)
    wsi = np.empty((NSSM, D, 2 * D), np_mm); bsi = np.empty((NSSM, 128, 8), np.float32)
    cwl = np.empty((NSSM, 128, KC, 3), np.float32)
    cbl = np.empty((NSSM, 128, KC), np.float32)
    wso = np.empty((NSSM, D, D), np_mm); bso = np.empty((NSSM, 1, D), np_mm)
    for i in range(NSSM):
        nw, nb = g["ssm_norm_w"][i], g["ssm_norm_b"][i]
        wsi[i] = (nw[:, None] * g["ssm_in_w"][i]).astype(np_mm)
        bsi[i] = fm_bias(nb @ g["ssm_in_w"][i] + g["ssm_in_b"][i])
        cwl[i] = g["ssm_conv_w"][i].reshape(KC, 128, 3).transpose(1, 0, 2)
        cbl[i] = g["ssm_conv_b"][i].reshape(KC, 128).T
        wso[i] = g["ssm_out_w"][i].astype(np_mm)
        bso[i] = g["ssm_out_b"][i][None, :].astype(np_mm)
    P.update(w_si=wsi, b_si=bsi, cw=cwl, cb=cbl, w_so=wso, b_so=bso)

    def rot_feat(w):
        """Apply rotate_half permutation (with sign) to output features of w [.., D]."""
        wh = w.reshape(w.shape[:-1] + (H, HD))
        out = np.empty_like(wh)
        out[..., 0:32] = -wh[..., 32:64]
        out[..., 32:64] = wh[..., 0:32]
        return out.reshape(w.shape)

    wq = np.empty((NATTN, D, D), np_mm); bq = np.empty((NATTN, 128, KC), np.float32)
    wk = np.empty((NATTN, D, D), np_mm); bk = np.empty((NATTN, 128, KC), np.float32)
    wqr = np.empty((NATTN, D, D), np_mm); bqr = np.empty((NATTN, 128, KC), np.float32)
    wkr = np.empty((NATTN, D, D), np_mm); bkr = np.empty((NATTN, 128, KC), np.float32)
    wv = np.empty((NATTN, D, D), np_mm); bv = np.empty((NATTN, 1, D), np_mm)
    wao = np.empty((NATTN, D, D), np_mm); bao = np.empty((NATTN, 1, D), np_mm)
    wf1 = np.empty((NATTN, D, FFD), np_mm)
    bf1 = np.empty((NATTN, 128, FFD // 128), np.float32)
    wf2 = np.empty((NATTN, FFD, D), np_mm); bf2 = np.empty((NATTN, 1, D), np_mm)
    for a in range(NATTN):
        n1w, n1b = g["a_ln1_w"][a], g["a_ln1_b"][a]
        wqf = n1w[:, None] * g["a_q_w"][a]
        bqf = n1b @ g["a_q_w"][a] + g["a_q_b"][a]
        wkf = n1w[:, None] * g["a_k_w"][a]
        bkf = n1b @ g["a_k_w"][a] + g["a_k_b"][a]
        wq[a] = wqf.astype(np_mm); bq[a] = fm_bias(bqf)
        wk[a] = wkf.astype(np_mm); bk[a] = fm_bias(bkf)
        wqr[a] = rot_feat(wqf).astype(np_mm); bqr[a] = fm_bias(rot_feat(bqf))
        wkr[a] = rot_feat(wkf).astype(np_mm); bkr[a] = fm_bias(rot_feat(bkf))
        wv[a] = (n1w[:, None] * g["a_v_w"][a]).astype(np_mm)
        bv[a] = (n1b @ g["a_v_w"][a] + g["a_v_b"][a])[None, :].astype(np_mm)
        wao[a] = g["a_o_w"][a].astype(np_mm)
        bao[a] = g["a_o_b"][a][None, :].astype(np_mm)
        n2w, n2b = g["a_ln2_w"][a], g["a_ln2_b"][a]
        wf1[a] = (n2w[:, None] * g["a_f1_w"][a]).astype(np_mm)
        bf1[a] = fm_bias(n2b @ g["a_f1_w"][a] + g["a_f1_b"][a])
        wf2[a] = g["a_f2_w"][a].astype(np_mm)
        bf2[a] = g["a_f2_b"][a][None, :].astype(np_mm)
    P.update(w_q=wq, b_q=bq, w_k=wk, b_k=bk, w_qr=wqr, b_qr=bqr,
             w_kr=wkr, b_kr=bkr, w_v=wv, b_v=bv, w_ao=wao, b_ao=bao,
             w_f1=wf1, b_f1=bf1, w_f2=wf2, b_f2=bf2)

    fnw, fnb = g["fn_w"], g["fn_b"]
    P["w_fr"] = (fnw[:, None] * g["frame_w"]).astype(np_mm)
    P["b_fr"] = (fnb @ g["frame_w"] + g["frame_b"])[None, :].astype(np_mm)
    P["w_sy"] = (fnw[:, None] * g["sym_w"]).astype(np_mm)
    P["b_sy"] = (fnb @ g["sym_w"] + g["sym_b"])[None, :].astype(np_mm)

    frames = g["frames"]
    inv_freq = 1.0 / (10000.0 ** (np.arange(0, HD, 2, dtype=np.float32) / HD))
    per_core = []
    for c in range(8):
        b, hhalf = c // 2, c % 2
        start = hhalf * 1024 - HALO
        idx = np.arange(start, start + T)
        inseq = (idx >= 0) & (idx < S)
        fr = np.zeros((T, F), np.float32)
        fr[inseq] = frames[b][idx[inseq]]
        freqs = np.outer(idx.astype(np.float32), inv_freq)
        cos32 = np.cos(freqs).astype(np.float32)   # [T, 32]
        sin32 = np.sin(freqs).astype(np.float32)
        pr = np.arange(128) % 32
        d = dict(P)
        d["framesT"] = np.ascontiguousarray(fr.T).astype(np_mm)
        d["smask"] = inseq.astype(np.float32)[None, :].astype(np_mm)
        d["cosB"] = np.ascontiguousarray(cos32[:, pr].T).astype(np_mm)
        d["sinB"] = np.ascontiguousarray(sin32[:, pr].T).astype(np_mm)
        per_core.append(d)
    return per_core


# ----------------------------------------------------------------- entry

def kernel(**inputs):
    dt_mm = os.environ.get("MJM_DT", "bfloat16")
    if dt_mm == "bfloat16":
        import ml_dtypes
        np_mm = ml_dtypes.bfloat16
    else:
        np_mm = np.float32

    trace = os.environ.get("MJM_TRACE", "0") == "1"
    if trace:
        import sys, types
        if "antenv.axon_hooks" not in sys.modules:
            try:
                from trn_agent_boot.trn_boot import _ntff_profile_via_ctypes
                hook = _ntff_profile_via_ctypes("/opt/axon/libaxon_pjrt.so")
                mod = types.ModuleType("antenv.axon_hooks")
                mod.get_axon_ntff_profile_hook = lambda: hook
                sys.modules["antenv.axon_hooks"] = mod
            except Exception:
                trace = False

    from concourse.bass_utils import run_bass_kernel_spmd

    key = ("nc", dt_mm)
    if key not in _CACHE:
        _CACHE[key] = _build_nc(dt_mm)
    nc = _CACHE[key]

    in_maps = _host_prep(inputs, np_mm)
    last_err = None
    res = None
    for _attempt in range(3):
        try:
            res = run_bass_kernel_spmd(nc, in_maps, core_ids=list(range(8)),
                                       trace=trace)
            break
        except Exception as e:                       # transient NRT/worker errors
            last_err = e
            import time as _time
            _time.sleep(5)
    if res is None:
        raise last_err

    if trace and res.exec_time_ns is not None:
        print(f"HW exec time: {res.exec_time_ns} ns")

    frame_out = np.zeros((B, S, F), np.float32)
    sym_out = np.zeros((B, S, V), np.float32)
    for c in range(8):
        b, hhalf = c // 2, c % 2
        sl = slice(hhalf * 1024, hhalf * 1024 + 1024)
        frame_out[b, sl] = res.results[c]["out_frame"][HALO:HALO + 1024]
        sym_out[b, sl] = res.results[c]["out_sym"][HALO:HALO + 1024]
    return frame_out, sym_out


# revision 30
# speedup vs baseline: 1.0250x; 1.0250x over previous
"""Trainium2 Bass kernel for MiniJMamba (12-layer SSM+attention hybrid).

Sharding: 8 cores = batch(4) x seq-half(2). Each core processes a
1152-token window (1024 local + 64-token halo each side, zero-padded at
sequence ends). The 10 depthwise convs each consume one token of halo
validity (10 <= 64). At the two attention layers, cores exchange K/V
halves with their pair core via AllGather over groups [2b, 2b+1].

Host-side prep folds every LayerNorm affine into the following matmul
weights, so on-device LN is pure normalization. Activations use two
layouts: token-major [128 tok, ...] for LN and feature-major [feat, tok]
as matmul lhsT; PE transposes bridge them. Matmul operands are bf16
(PSUM accumulation fp32); the residual stream and LN stats stay fp32.
"""

import os
import numpy as np

B, S, F, D, H, V = 4, 2048, 512, 512, 8, 1024
HD, FFD = 64, 2048
NSSM, NATTN = 10, 2
ATTN_POS = (4, 8)
EPS = 1e-5
HALO = 64
T = 1024 + 2 * HALO            # 1152 tokens per core window
NT = T // 128                  # 9 token tiles
KC = D // 128                  # 4 feature chunks
PANELS = [(0, 512), (512, 512), (1024, T - 1024)]  # free-dim panels of T
# attention query window: local tokens + 6-token halo each side (enough for
# the convolutions remaining after each attention layer)
QPANELS = [(58, 512), (570, 512), (1082, 12)]
NKV = S // 128                 # 16 key chunks (full sequence)

_CACHE = {}


# ----------------------------------------------------------------- build

def _build_nc(dt_mm_name="bfloat16"):
    import concourse.bass as bass
    import concourse.tile as tile
    from concourse import bacc, mybir
    from concourse.masks import make_identity
    import contextlib

    F32 = mybir.dt.float32
    DTM = getattr(mybir.dt, dt_mm_name)
    AF = mybir.ActivationFunctionType
    ALU = mybir.AluOpType
    ts = bass.ts

    nc = bacc.Bacc(None, target_bir_lowering=False, num_devices=8)

    def din(name, shape, dt=DTM):
        return nc.dram_tensor(name, shape, dt, kind="ExternalInput")

    # ---- inputs
    framesT = din("framesT", [D, T])
    smask = din("smask", [1, T])
    cosB = din("cosB", [128, T])
    sinB = din("sinB", [128, T])
    w_in = din("w_in", [F, D]); b_in = din("b_in", [1, D])
    w_si = din("w_si", [NSSM, D, 2 * D]); b_si = din("b_si", [NSSM, 128, 8], F32)
    cw = din("cw", [NSSM, 128, KC, 3], F32); cb = din("cb", [NSSM, 128, KC], F32)
    w_so = din("w_so", [NSSM, D, D]); b_so = din("b_so", [NSSM, 1, D])
    w_q = din("w_q", [NATTN, D, D]); b_q = din("b_q", [NATTN, 128, KC], F32)
    w_k = din("w_k", [NATTN, D, D]); b_k = din("b_k", [NATTN, 128, KC], F32)
    w_qr = din("w_qr", [NATTN, D, D]); b_qr = din("b_qr", [NATTN, 128, KC], F32)
    w_kr = din("w_kr", [NATTN, D, D]); b_kr = din("b_kr", [NATTN, 128, KC], F32)
    w_v = din("w_v", [NATTN, D, D]); b_v = din("b_v", [NATTN, 1, D])
    w_ao = din("w_ao", [NATTN, D, D]); b_ao = din("b_ao", [NATTN, 1, D])
    w_f1 = din("w_f1", [NATTN, D, FFD]); b_f1 = din("b_f1", [NATTN, 128, FFD // 128], F32)
    w_f2 = din("w_f2", [NATTN, FFD, D]); b_f2 = din("b_f2", [NATTN, 1, D])
    w_fr = din("w_fr", [D, F]); b_fr = din("b_fr", [1, F])
    w_sy = din("w_sy", [D, V]); b_sy = din("b_sy", [1, V])

    out_frame = nc.dram_tensor("out_frame", [T, F], F32, kind="ExternalOutput")
    out_sym = nc.dram_tensor("out_sym", [T, V], F32, kind="ExternalOutput")

    RG = [[0, 1], [2, 3], [4, 5], [6, 7]]

    with tile.TileContext(nc) as tc:
        ctx = contextlib.ExitStack()
        with ctx:
            persist = ctx.enter_context(tc.tile_pool(name="persist", bufs=1))
            wpool = ctx.enter_context(tc.tile_pool(name="wpool", bufs=2))
            act = ctx.enter_context(tc.tile_pool(name="act", bufs=1))
            big = ctx.enter_context(tc.tile_pool(name="big", bufs=1))
            sm2 = ctx.enter_context(tc.tile_pool(name="sm2", bufs=2))
            sm1 = ctx.enter_context(tc.tile_pool(name="sm1", bufs=1))
            wpool1 = ctx.enter_context(tc.tile_pool(name="wpool1", bufs=2))
            sm3 = ctx.enter_context(tc.tile_pool(name="sm3", bufs=3))
            ps_mm = ctx.enter_context(tc.tile_pool(name="ps_mm", bufs=3, space="PSUM"))
            ps_av = ctx.enter_context(tc.tile_pool(name="ps_av", bufs=3, space="PSUM"))
            ps_tr = ctx.enter_context(tc.tile_pool(name="ps_tr", bufs=2, space="PSUM"))
            dram = ctx.enter_context(tc.tile_pool(name="dram", bufs=2, space="DRAM"))

            ident = persist.tile([128, 128], DTM)
            make_identity(nc, ident)
            ones_f32 = persist.tile([1, 64], F32)
            nc.vector.memset(ones_f32[:], 1.0)
            epst = persist.tile([128, 1], F32)
            nc.vector.memset(epst[:], EPS)
            zerob = persist.tile([128, 1], F32)
            nc.vector.memset(zerob[:], 0.0)

            smask_bc = persist.tile([128, T], DTM)
            nc.sync.dma_start(smask_bc[:], smask[:].to_broadcast((128, T)))
            cosB_sb = persist.tile([128, T], DTM)
            nc.sync.dma_start(cosB_sb[:], cosB[:])
            sinB_sb = persist.tile([128, T], DTM)
            nc.sync.dma_start(sinB_sb[:], sinB[:])

            x = persist.tile([128, NT, D], F32)     # residual stream, token-major

            # ---------- helpers ----------
            def load_w(w2d, kchunks, n_out, tag):
                pool_ = wpool1 if tag == "w_d_d2" else wpool
                t_ = pool_.tile([128, kchunks, n_out], DTM, tag=tag)
                nc.sync.dma_start(t_[:], w2d.rearrange("(kc p) n -> p kc n", p=128))
                return t_

            def load_bias_fm(b2d, ncols, tag):
                t_ = wpool.tile([128, ncols], F32, tag=tag)
                nc.sync.dma_start(t_[:], b2d)
                return t_

            def load_bias_bc(b2d, n, tag):
                t_ = wpool.tile([128, n], DTM, tag=tag)
                nc.sync.dma_start(t_[:], b2d.to_broadcast((128, n)))
                return t_

            def mm_fm(out_fm, actT, w_sb, bias_fm, nchunks, epi_func, off=0):
                """out_fm[:, off+n, :] = epi(w-cols.T @ actT-chunks + bias)."""
                for n in range(nchunks):
                    for (qo, qw) in PANELS:
                        ps = ps_mm.tile([128, 512], F32, tag="ps_mm")
                        for kc_ in range(KC):
                            nc.tensor.matmul(
                                ps[:, :qw], lhsT=w_sb[:, kc_, ts(off + n, 128)],
                                rhs=actT[:, kc_, qo:qo + qw],
                                start=(kc_ == 0), stop=(kc_ == KC - 1))
                        nc.scalar.activation(
                            out=out_fm[:, off + n, qo:qo + qw], in_=ps[:, :qw],
                            func=epi_func, bias=bias_fm[:, off + n:off + n + 1])

            def mm_tm(actT, kchunks, w_sb, n_out, consumer):
                """token-major out: per t-tile psum [128, n<=512] -> consumer
                (bias is applied by the consumer, off the PE)."""
                nh = (n_out + 511) // 512
                for t_ in range(NT):
                    for nh_i in range(nh):
                        n0 = nh_i * 512
                        nw = min(512, n_out - n0)
                        ps = ps_mm.tile([128, 512], F32, tag="ps_mm")
                        for kc_ in range(kchunks):
                            nc.tensor.matmul(
                                ps[:, :nw], lhsT=actT[:, kc_, ts(t_, 128)],
                                rhs=w_sb[:, kc_, n0:n0 + nw],
                                start=(kc_ == 0), stop=(kc_ == kchunks - 1))
                        consumer(t_, ps[:, :nw], n0, nw)

            def layernorm_T(xt):
                """Pure-normalize x (token-major fp32) -> transposed [128, KC, T] bf16.

                Fully per-tile pipeline: tile t's transposes (PE) overlap tile
                t+1's stats (DVE) instead of a batched-stats barrier."""
                xhatT = act.tile([128, KC, T], DTM, tag="xhatT", name="xhatT")
                for t_ in range(NT):
                    st6 = sm3.tile([128, 6], F32, tag="ln_st6", name="ln_st6")
                    mv = sm3.tile([128, 2], F32, tag="ln_mv", name="ln_mv")
                    rstd = sm3.tile([128, 1], F32, tag="ln_rstd", name="ln_rstd")
                    nc.vector.bn_stats(st6[:], xt[:, t_, :])
                    nc.vector.bn_aggr(mv[:], st6[:])
                    nc.scalar.activation(out=rstd[:], in_=mv[:, 1:2], func=AF.Sqrt,
                                         bias=epst[:])
                    nc.vector.reciprocal(rstd[:], rstd[:])
                    xh = sm2.tile([128, D], DTM, tag="ln_xh", name="ln_xh")
                    nc.vector.tensor_scalar(
                        out=xh[:], in0=xt[:, t_, :],
                        scalar1=mv[:, 0:1], scalar2=rstd[:, 0:1],
                        op0=ALU.subtract, op1=ALU.mult)
                    pt = ps_tr.tile([128, 512], DTM, tag="ps_tr", name="pt")
                    for kc_ in range(KC):
                        nc.tensor.transpose(pt[:, ts(kc_, 128)],
                                            xh[:, ts(kc_, 128)], ident[:])
                    nc.vector.tensor_copy(
                        xhatT[:, :, ts(t_, 128)],
                        pt[:].rearrange("p (c q) -> p c q", q=128))
                return xhatT

            def rope_combine(qk, qkr):
                """qk <- qk*cosB + qkr*sinB (rotation term computed via folded weights)."""
                for c in range(KC):
                    tmp = sm1.tile([128, T], F32, tag="tmp_a", name="rope_tmp")
                    sn = sm1.tile([128, T], F32, tag="tmp_b", name="rope_s")
                    nc.vector.tensor_mul(tmp[:], qk[:, c, :], cosB_sb[:])
                    nc.gpsimd.tensor_mul(sn[:], qkr[:, c, :], sinB_sb[:])
                    nc.vector.tensor_add(qk[:, c, :], tmp[:], sn[:])

            # ---------- input projection ----------
            framesT_sb = act.tile([128, KC, T], DTM, tag="xhatT", name="framesT_sb")
            nc.sync.dma_start(framesT_sb[:],
                              framesT[:].rearrange("(kc p) t -> p kc t", p=128))
            w_in_sb = load_w(w_in[:], KC, D, "w_d_d")
            b_in_sb = load_bias_bc(b_in[:], D, "b_row_d")

            def into_x(t_, ps, n0, nw):
                nc.vector.tensor_add(x[:, t_, n0:n0 + nw], ps, b_in_sb[:, n0:n0 + nw])
            mm_tm(framesT_sb, KC, w_in_sb, D, into_x)

            # ---------- layers ----------
            si = ai = 0
            for layer in range(NSSM + NATTN):
                if layer in ATTN_POS:
                    a = ai; ai += 1
                    # === attention block ===
                    xhatT = layernorm_T(x)

                    # K projection first so the exchange starts early
                    w_k_sb = load_w(w_k[a], KC, D, "w_d_d")
                    b_k_sb = load_bias_fm(b_k[a], KC, "b_fm_4")
                    k_fm = act.tile([128, KC, T], DTM, tag="mix2", name="k_fm")
                    mm_fm(k_fm, xhatT, w_k_sb, b_k_sb, KC, AF.Identity)
                    w_kr_sb = load_w(w_kr[a], KC, D, "w_d_d")
                    b_kr_sb = load_bias_fm(b_kr[a], KC, "b_fm_4")
                    kr_fm = act.tile([128, KC, T], DTM, tag="rot", name="kr_fm")
                    mm_fm(kr_fm, xhatT, w_kr_sb, b_kr_sb, KC, AF.Identity)
                    rope_combine(k_fm, kr_fm)
                    k_bin = dram.tile([D, 1024], DTM, tag="k_bin", name="k_bin")
                    nc.sync.dma_start(
                        k_bin[:].rearrange("(c p) t -> p c t", p=128),
                        k_fm[:, :, HALO:HALO + 1024])
                    k_bout = dram.tile([2 * D, 1024], DTM, tag="k_bout", name="k_bout")
                    nc.gpsimd.collective_compute(
                        "AllGather", ALU.bypass, replica_groups=RG,
                        ins=[k_bin.opt()], outs=[k_bout.opt()])
                    k_full = big.tile([128, KC, S], DTM, tag="k_full", name="k_full")
                    nc.sync.dma_start(
                        k_full[:, :, 0:1024],
                        k_bout[0:D].rearrange("(c p) t -> p c t", p=128))
                    nc.sync.dma_start(
                        k_full[:, :, 1024:2048],
                        k_bout[D:2 * D].rearrange("(c p) t -> p c t", p=128))

                    # V projection (token-major) -> exchange
                    w_v_sb = load_w(w_v[a], KC, D, "w_d_d")
                    b_v_sb = load_bias_bc(b_v[a], D, "b_row_d")
                    v_loc = act.tile([128, NT, D], DTM, tag="mix1", name="v_loc")

                    def into_v(t_, ps, n0, nw):
                        nc.vector.tensor_add(v_loc[:, t_, n0:n0 + nw], ps,
                                             b_v_sb[:, n0:n0 + nw])
                    mm_tm(xhatT, KC, w_v_sb, D, into_v)
                    # exchange V in two feature halves so heads 0-3 can
                    # start their PV matmuls after only the first collective
                    v_tm = big.tile([128, NKV, H, HD + 1], DTM, tag="v_tm", name="v_tm")
                    for vh in range(2):
                        c0 = vh * 256
                        v_bin = dram.tile([1024, 256], DTM, tag="v_bin", name="v_bin")
                        nc.sync.dma_start(v_bin[0:64], v_loc[64:128, 0, c0:c0 + 256])
                        nc.sync.dma_start(
                            v_bin[64:960].rearrange("(c p) n -> p c n", p=128),
                            v_loc[:, 1:8, c0:c0 + 256])
                        nc.sync.dma_start(v_bin[960:1024], v_loc[0:64, 8, c0:c0 + 256])
                        v_bout = dram.tile([S, 256], DTM, tag="v_bout", name="v_bout")
                        nc.gpsimd.collective_compute(
                            "AllGather", ALU.bypass, replica_groups=RG,
                            ins=[v_bin.opt()], outs=[v_bout.opt()])
                        for h_ in range(4):
                            nc.sync.dma_start(
                                v_tm[:, :, vh * 4 + h_, 0:HD],
                                v_bout[:, h_ * HD:(h_ + 1) * HD].rearrange(
                                    "(kc p) d -> p kc d", p=128))
                    nc.vector.memset(v_tm[:, :, :, HD:HD + 1], 1.0)

                    # Q projection (overlaps the collectives)
                    w_q_sb = load_w(w_q[a], KC, D, "w_d_d")
                    b_q_sb = load_bias_fm(b_q[a], KC, "b_fm_4")
                    q_fm = act.tile([128, KC, T], DTM, tag="q_fm", name="q_fm")
                    mm_fm(q_fm, xhatT, w_q_sb, b_q_sb, KC, AF.Identity)
                    w_qr_sb = load_w(w_qr[a], KC, D, "w_d_d")
                    b_qr_sb = load_bias_fm(b_qr[a], KC, "b_fm_4")
                    qr_fm = act.tile([128, KC, T], DTM, tag="rot", name="qr_fm")
                    mm_fm(qr_fm, xhatT, w_qr_sb, b_qr_sb, KC, AF.Identity)
                    rope_combine(q_fm, qr_fm)

                    # attention core: q restricted to the columns whose
                    # outputs are still needed (local + remaining conv halo)
                    ctx_fm = act.tile([128, KC, T], DTM, tag="mix1", name="ctx_fm")
                    nc.vector.memset(ctx_fm[:], 0.0)
                    for h in range(H):
                        r0 = (h % 2) * 64
                        ch = h // 2
                        av_tiles = [ps_av.tile([65, 512], F32, tag="ps_av", name="av")
                                    for _ in PANELS]
                        for c in range(NKV):
                            ex = sm3.tile([128, T], DTM, tag="ebuf", name="ex")
                            for j, (qo, qw) in enumerate(QPANELS):
                                ps = ps_mm.tile([128, 512], F32, tag="ps_mm", name="sc")
                                nc.tensor.matmul(
                                    ps[:, :qw],
                                    lhsT=k_full[r0:r0 + 64, ch, ts(c, 128)],
                                    rhs=q_fm[r0:r0 + 64, ch, qo:qo + qw],
                                    start=True, stop=True)
                                nc.scalar.activation(
                                    out=ex[:, qo:qo + qw], in_=ps[:, :qw],
                                    func=AF.Exp, scale=0.125)
                            for j, (qo, qw) in enumerate(QPANELS):
                                nc.tensor.matmul(
                                    av_tiles[j][:, :qw],
                                    lhsT=v_tm[:, c, h, :],
                                    rhs=ex[:, qo:qo + qw],
                                    start=(c == 0), stop=(c == NKV - 1))
                        inv = sm1.tile([1, T], F32, tag="attn_inv", name="inv")
                        invb = sm1.tile([64, T], F32, tag="attn_invb", name="invb")
                        for j, (qo, qw) in enumerate(QPANELS):
                            nc.vector.reciprocal(inv[:, qo:qo + qw],
                                                 av_tiles[j][64:65, :qw])
                            ib_ps = ps_tr.tile([64, 512], F32, tag="ps_tr", name="ib")
                            nc.tensor.matmul(ib_ps[:, :qw],
                                             lhsT=ones_f32[:],
                                             rhs=inv[:, qo:qo + qw],
                                             start=True, stop=True)
                            nc.vector.tensor_copy(invb[:, qo:qo + qw], ib_ps[:, :qw])
                        if r0 == 0:
                            for j, (qo, qw) in enumerate(QPANELS):
                                nc.vector.tensor_mul(
                                    ctx_fm[0:64, ch, qo:qo + qw],
                                    av_tiles[j][0:64, :qw], invb[:, qo:qo + qw])
                        else:
                            stage = sm2.tile([64, T], DTM, tag="ctx_stage", name="cst")
                            for j, (qo, qw) in enumerate(QPANELS):
                                nc.vector.tensor_mul(
                                    stage[:, qo:qo + qw],
                                    av_tiles[j][0:64, :qw], invb[:, qo:qo + qw])
                            nc.sync.dma_start(ctx_fm[64:128, ch, :], stage[:])

                    # output projection + residual
                    w_ao_sb = load_w(w_ao[a], KC, D, "w_d_d")
                    b_ao_sb = load_bias_bc(b_ao[a], D, "b_row_d")

                    def into_res(t_, ps, n0, nw):
                        nc.vector.tensor_add(x[:, t_, n0:n0 + nw],
                                             x[:, t_, n0:n0 + nw], ps)
                        nc.gpsimd.tensor_add(x[:, t_, n0:n0 + nw],
                                             x[:, t_, n0:n0 + nw],
                                             b_ao_sb[:, n0:n0 + nw])
                    mm_tm(ctx_fm, KC, w_ao_sb, D, into_res)

                    # === FF block (quarters over the 2048 hidden dim,
                    # each quarter's f2 partial sum added straight into x) ===
                    xhatT = layernorm_T(x)
                    for hf in range(4):
                        w_f1_sb = wpool.tile([128, KC, 512], DTM, tag="w_d_d",
                                             name="w_f1_sb")
                        nc.sync.dma_start(
                            w_f1_sb[:],
                            w_f1[a][:, hf * 512:(hf + 1) * 512].rearrange(
                                "(kc p) n -> p kc n", p=128))
                        b_f1_sb = wpool.tile([128, 4], F32, tag="b_fm_4", name="b_f1_sb")
                        nc.sync.dma_start(b_f1_sb[:], b_f1[a][:, hf * 4:(hf + 1) * 4])
                        g_fm = act.tile([128, 4, T], DTM, tag="mix1", name="g_fm")
                        mm_fm(g_fm, xhatT, w_f1_sb, b_f1_sb, 4, AF.Gelu)
                        w_f2_sb = wpool.tile([128, 4, D], DTM, tag="w_d_d",
                                             name="w_f2_sb")
                        nc.sync.dma_start(
                            w_f2_sb[:],
                            w_f2[a][hf * 512:(hf + 1) * 512].rearrange(
                                "(kc p) n -> p kc n", p=128))
                        if hf == 0:
                            b_f2_bc = load_bias_bc(b_f2[a], D, "b_row_d")

                            def into_ffres(t_, ps, n0, nw):
                                nc.vector.tensor_add(x[:, t_, n0:n0 + nw],
                                                     x[:, t_, n0:n0 + nw], ps)
                                nc.gpsimd.tensor_add(x[:, t_, n0:n0 + nw],
                                                     x[:, t_, n0:n0 + nw],
                                                     b_f2_bc[:, n0:n0 + nw])
                        else:
                            def into_ffres(t_, ps, n0, nw):
                                nc.vector.tensor_add(x[:, t_, n0:n0 + nw],
                                                     x[:, t_, n0:n0 + nw], ps)
                        mm_tm(g_fm, KC, w_f2_sb, D, into_ffres)
                else:
                    i = si; si += 1
                    # === SSM block ===
                    xhatT = layernorm_T(x)
                    w_si_sb = load_w(w_si[i], KC, 2 * D, "w_d_d2")
                    b_si_sb = load_bias_fm(b_si[i], 8, "b_fm_8")
                    z_fm = act.tile([128, 8, T], DTM, tag="g_fm", name="z_fm")
                    mm_fm(z_fm, xhatT, w_si_sb, b_si_sb, 4, AF.Sigmoid, off=0)
                    mm_fm(z_fm, xhatT, w_si_sb, b_si_sb, 4, AF.Identity, off=4)
                    # gating + seq mask; cand buffer has a zero column each side
                    cand = act.tile([128, KC, T + 2], DTM, tag="mix2", name="cand")
                    nc.vector.memset(cand[:, :, 0:1], 0.0)
                    nc.vector.memset(cand[:, :, T + 1:T + 2], 0.0)
                    cw_sb = wpool.tile([128, KC, 3], F32, tag="cw", name="cw_sb")
                    nc.sync.dma_start(cw_sb[:], cw[i])
                    cb_sb = wpool.tile([128, KC], F32, tag="cb", name="cb_sb")
                    nc.sync.dma_start(cb_sb[:], cb[i])
                    conv_fm = act.tile([128, KC, T], DTM, tag="mix1", name="conv_fm")
                    # process gating+conv in T-halves so the out-projection of
                    # the first token tiles overlaps the second half's conv
                    for (g0, g1), (o0, o1) in (((0, 641), (0, 640)),
                                               ((641, T), (640, T))):
                        for c in range(KC):
                            sg = sm3.tile([128, T], DTM, tag="ebuf", name="sg")
                            nc.vector.tensor_mul(sg[:, g0:g1], z_fm[:, c, g0:g1],
                                                 z_fm[:, KC + c, g0:g1])
                            nc.gpsimd.tensor_mul(cand[:, c, 1 + g0:1 + g1],
                                                 sg[:, g0:g1], smask_bc[:, g0:g1])
                        wd = o1 - o0
                        for c in range(KC):
                            t1 = sm2.tile([128, T], DTM, tag="ctmp_a", name="conv_t1")
                            t2 = sm2.tile([128, T], DTM, tag="ctmp_b", name="conv_t2")
                            nc.vector.tensor_scalar_mul(t1[:, :wd],
                                                        cand[:, c, o0:o1],
                                                        cw_sb[:, c, 0:1])
                            nc.vector.scalar_tensor_tensor(
                                out=t2[:, :wd], in0=cand[:, c, o0 + 1:o1 + 1],
                                scalar=cw_sb[:, c, 1:2], in1=t1[:, :wd],
                                op0=ALU.mult, op1=ALU.add)
                            nc.vector.scalar_tensor_tensor(
                                out=t1[:, :wd], in0=cand[:, c, o0 + 2:o1 + 2],
                                scalar=cw_sb[:, c, 2:3], in1=t2[:, :wd],
                                op0=ALU.mult, op1=ALU.add)
                            nc.scalar.activation(
                                out=conv_fm[:, c, o0:o1], in_=t1[:, :wd],
                                func=AF.Relu, bias=cb_sb[:, c:c + 1])
                    w_so_sb = load_w(w_so[i], KC, D, "w_d_d")
                    b_so_sb = load_bias_bc(b_so[i], D, "b_row_d")

                    def into_res_s(t_, ps, n0, nw):
                        nc.vector.tensor_add(x[:, t_, n0:n0 + nw],
                                             x[:, t_, n0:n0 + nw], ps)
                        nc.gpsimd.tensor_add(x[:, t_, n0:n0 + nw],
                                             x[:, t_, n0:n0 + nw],
                                             b_so_sb[:, n0:n0 + nw])
                    mm_tm(conv_fm, KC, w_so_sb, D, into_res_s)

            # ---------- heads ----------
            hT = layernorm_T(x)
            w_fr_sb = load_w(w_fr[:], KC, F, "w_d_d")
            b_fr_sb = load_bias_bc(b_fr[:], F, "b_row_d")

            def into_frame(t_, ps, n0, nw):
                st_ = sm2.tile([128, 512], F32, tag="ln_xh", name="ost")
                nc.vector.tensor_add(st_[:, :nw], ps, b_fr_sb[:, n0:n0 + nw])
                nc.sync.dma_start(out_frame[ts(t_, 128), n0:n0 + nw], st_[:, :nw])
            mm_tm(hT, KC, w_fr_sb, F, into_frame)

            w_sy_sb = wpool1.tile([128, KC, V], DTM, tag="w_d_d2", name="w_sy_sb")
            nc.sync.dma_start(w_sy_sb[:],
                              w_sy[:].rearrange("(kc p) n -> p kc n", p=128))
            b_sy_sb = wpool1.tile([128, V], DTM, tag="b_row_2d", name="b_sy_sb")
            nc.sync.dma_start(b_sy_sb[:], b_sy[:].to_broadcast((128, V)))

            def into_sym(t_, ps, n0, nw):
                st_ = sm2.tile([128, 512], F32, tag="ln_xh", name="ost")
                nc.vector.tensor_add(st_[:, :nw], ps, b_sy_sb[:, n0:n0 + nw])
                nc.sync.dma_start(out_sym[ts(t_, 128), n0:n0 + nw], st_[:, :nw])
            mm_tm(hT, KC, w_sy_sb, V, into_sym)

    nc.finalize()
    return nc


# ------------------------------------------------------------- host prep

def _host_prep(inputs, np_mm):
    g = {k: np.asarray(v, np.float32) if np.asarray(v).dtype != np.bool_
         else np.asarray(v) for k, v in inputs.items()}

    def fm_bias(b):      # [N] -> [128, N//128]
        return np.ascontiguousarray(b.reshape(-1, 128).T).astype(np.float32)

    P = {}
    P["w_in"] = g["input_w"].astype(np_mm)
    P["b_in"] = g["input_b"][None, :].astype(np_mm)
    wsi = np.empty((NSSM, D, 2 * D), np_mm); bsi = np.empty((NSSM, 128, 8), np.float32)
    cwl = np.empty((NSSM, 128, KC, 3), np.float32)
    cbl = np.empty((NSSM, 128, KC), np.float32)
    wso = np.empty((NSSM, D, D), np_mm); bso = np.empty((NSSM, 1, D), np_mm)
    for i in range(NSSM):
        nw, nb = g["ssm_norm_w"][i], g["ssm_norm_b"][i]
        wsi[i] = (nw[:, None] * g["ssm_in_w"][i]).astype(np_mm)
        bsi[i] = fm_bias(nb @ g["ssm_in_w"][i] + g["ssm_in_b"][i])
        cwl[i] = g["ssm_conv_w"][i].reshape(KC, 128, 3).transpose(1, 0, 2)
        cbl[i] = g["ssm_conv_b"][i].reshape(KC, 128).T
        wso[i] = g["ssm_out_w"][i].astype(np_mm)
        bso[i] = g["ssm_out_b"][i][None, :].astype(np_mm)
    P.update(w_si=wsi, b_si=bsi, cw=cwl, cb=cbl, w_so=wso, b_so=bso)

    def rot_feat(w):
        """Apply rotate_half permutation (with sign) to output features of w [.., D]."""
        wh = w.reshape(w.shape[:-1] + (H, HD))
        out = np.empty_like(wh)
        out[..., 0:32] = -wh[..., 32:64]
        out[..., 32:64] = wh[..., 0:32]
        return out.reshape(w.shape)

    wq = np.empty((NATTN, D, D), np_mm); bq = np.empty((NATTN, 128, KC), np.float32)
    wk = np.empty((NATTN, D, D), np_mm); bk = np.empty((NATTN, 128, KC), np.float32)
    wqr = np.empty((NATTN, D, D), np_mm); bqr = np.empty((NATTN, 128, KC), np.float32)
    wkr = np.empty((NATTN, D, D), np_mm); bkr = np.empty((NATTN, 128, KC), np.float32)
    wv = np.empty((NATTN, D, D), np_mm); bv = np.empty((NATTN, 1, D), np_mm)
    wao = np.empty((NATTN, D, D), np_mm); bao = np.empty((NATTN, 1, D), np_mm)
    wf1 = np.empty((NATTN, D, FFD), np_mm)
    bf1 = np.empty((NATTN, 128, FFD // 128), np.float32)
    wf2 = np.empty((NATTN, FFD, D), np_mm); bf2 = np.empty((NATTN, 1, D), np_mm)
    for a in range(NATTN):
        n1w, n1b = g["a_ln1_w"][a], g["a_ln1_b"][a]
        wqf = n1w[:, None] * g["a_q_w"][a]
        bqf = n1b @ g["a_q_w"][a] + g["a_q_b"][a]
        wkf = n1w[:, None] * g["a_k_w"][a]
        bkf = n1b @ g["a_k_w"][a] + g["a_k_b"][a]
        wq[a] = wqf.astype(np_mm); bq[a] = fm_bias(bqf)
        wk[a] = wkf.astype(np_mm); bk[a] = fm_bias(bkf)
        wqr[a] = rot_feat(wqf).astype(np_mm); bqr[a] = fm_bias(rot_feat(bqf))
        wkr[a] = rot_feat(wkf).astype(np_mm); bkr[a] = fm_bias(rot_feat(bkf))
        wv[a] = (n1w[:, None] * g["a_v_w"][a]).astype(np_mm)
        bv[a] = (n1b @ g["a_v_w"][a] + g["a_v_b"][a])[None, :].astype(np_mm)
        wao[a] = g["a_o_w"][a].astype(np_mm)
        bao[a] = g["a_o_b"][a][None, :].astype(np_mm)
        n2w, n2b = g["a_ln2_w"][a], g["a_ln2_b"][a]
        wf1[a] = (n2w[:, None] * g["a_f1_w"][a]).astype(np_mm)
        bf1[a] = fm_bias(n2b @ g["a_f1_w"][a] + g["a_f1_b"][a])
        wf2[a] = g["a_f2_w"][a].astype(np_mm)
        bf2[a] = g["a_f2_b"][a][None, :].astype(np_mm)
    P.update(w_q=wq, b_q=bq, w_k=wk, b_k=bk, w_qr=wqr, b_qr=bqr,
             w_kr=wkr, b_kr=bkr, w_v=wv, b_v=bv, w_ao=wao, b_ao=bao,
             w_f1=wf1, b_f1=bf1, w_f2=wf2, b_f2=bf2)

    fnw, fnb = g["fn_w"], g["fn_b"]
    P["w_fr"] = (fnw[:, None] * g["frame_w"]).astype(np_mm)
    P["b_fr"] = (fnb @ g["frame_w"] + g["frame_b"])[None, :].astype(np_mm)
    P["w_sy"] = (fnw[:, None] * g["sym_w"]).astype(np_mm)
    P["b_sy"] = (fnb @ g["sym_w"] + g["sym_b"])[None, :].astype(np_mm)

    frames = g["frames"]
    inv_freq = 1.0 / (10000.0 ** (np.arange(0, HD, 2, dtype=np.float32) / HD))
    per_core = []
    for c in range(8):
        b, hhalf = c // 2, c % 2
        start = hhalf * 1024 - HALO
        idx = np.arange(start, start + T)
        inseq = (idx >= 0) & (idx < S)
        fr = np.zeros((T, F), np.float32)
        fr[inseq] = frames[b][idx[inseq]]
        freqs = np.outer(idx.astype(np.float32), inv_freq)
        cos32 = np.cos(freqs).astype(np.float32)   # [T, 32]
        sin32 = np.sin(freqs).astype(np.float32)
        pr = np.arange(128) % 32
        d = dict(P)
        d["framesT"] = np.ascontiguousarray(fr.T).astype(np_mm)
        d["smask"] = inseq.astype(np.float32)[None, :].astype(np_mm)
        d["cosB"] = np.ascontiguousarray(cos32[:, pr].T).astype(np_mm)
        d["sinB"] = np.ascontiguousarray(sin32[:, pr].T).astype(np_mm)
        per_core.append(d)
    return per_core


# ----------------------------------------------------------------- entry

def kernel(**inputs):
    dt_mm = os.environ.get("MJM_DT", "bfloat16")
    if dt_mm == "bfloat16":
        import ml_dtypes
        np_mm = ml_dtypes.bfloat16
    else:
        np_mm = np.float32

    trace = os.environ.get("MJM_TRACE", "0") == "1"
    if trace:
        import sys, types
        if "antenv.axon_hooks" not in sys.modules:
            try:
                from trn_agent_boot.trn_boot import _ntff_profile_via_ctypes
                hook = _ntff_profile_via_ctypes("/opt/axon/libaxon_pjrt.so")
                mod = types.ModuleType("antenv.axon_hooks")
                mod.get_axon_ntff_profile_hook = lambda: hook
                sys.modules["antenv.axon_hooks"] = mod
            except Exception:
                trace = False

    from concourse.bass_utils import run_bass_kernel_spmd

    key = ("nc", dt_mm)
    if key not in _CACHE:
        _CACHE[key] = _build_nc(dt_mm)
    nc = _CACHE[key]

    in_maps = _host_prep(inputs, np_mm)
    last_err = None
    res = None
    for _attempt in range(3):
        try:
            res = run_bass_kernel_spmd(nc, in_maps, core_ids=list(range(8)),
                                       trace=trace)
            break
        except Exception as e:                       # transient NRT/worker errors
            last_err = e
            import time as _time
            _time.sleep(5)
    if res is None:
        raise last_err

    if trace and res.exec_time_ns is not None:
        print(f"HW exec time: {res.exec_time_ns} ns")

    frame_out = np.zeros((B, S, F), np.float32)
    sym_out = np.zeros((B, S, V), np.float32)
    for c in range(8):
        b, hhalf = c // 2, c % 2
        sl = slice(hhalf * 1024, hhalf * 1024 + 1024)
        frame_out[b, sl] = res.results[c]["out_frame"][HALO:HALO + 1024]
        sym_out[b, sl] = res.results[c]["out_sym"][HALO:HALO + 1024]
    return frame_out, sym_out


# revision 31
# speedup vs baseline: 1.0513x; 1.0257x over previous
"""Trainium2 Bass kernel for MiniJMamba (12-layer SSM+attention hybrid).

Sharding: 8 cores = batch(4) x seq-half(2). Each core processes a
1152-token window (1024 local + 64-token halo each side, zero-padded at
sequence ends). The 10 depthwise convs each consume one token of halo
validity (10 <= 64). At the two attention layers, cores exchange K/V
halves with their pair core via AllGather over groups [2b, 2b+1].

Host-side prep folds every LayerNorm affine into the following matmul
weights, so on-device LN is pure normalization. Activations use two
layouts: token-major [128 tok, ...] for LN and feature-major [feat, tok]
as matmul lhsT; PE transposes bridge them. Matmul operands are bf16
(PSUM accumulation fp32); the residual stream and LN stats stay fp32.
"""

import os
import numpy as np

B, S, F, D, H, V = 4, 2048, 512, 512, 8, 1024
HD, FFD = 64, 2048
NSSM, NATTN = 10, 2
ATTN_POS = (4, 8)
EPS = 1e-5
HALO = 64
T = 1024 + 2 * HALO            # 1152 tokens per core window
NT = T // 128                  # 9 token tiles
KC = D // 128                  # 4 feature chunks
PANELS = [(0, 512), (512, 512), (1024, T - 1024)]  # free-dim panels of T
# attention query window: local tokens + 6-token halo each side (enough for
# the convolutions remaining after each attention layer)
QPANELS = [(58, 512), (570, 512), (1082, 12)]
NKV = S // 128                 # 16 key chunks (full sequence)

_CACHE = {}


# ----------------------------------------------------------------- build

def _build_nc(dt_mm_name="bfloat16"):
    import concourse.bass as bass
    import concourse.tile as tile
    from concourse import bacc, mybir
    from concourse.masks import make_identity
    import contextlib

    F32 = mybir.dt.float32
    DTM = getattr(mybir.dt, dt_mm_name)
    AF = mybir.ActivationFunctionType
    ALU = mybir.AluOpType
    ts = bass.ts

    nc = bacc.Bacc(None, target_bir_lowering=False, num_devices=8)

    def din(name, shape, dt=DTM):
        return nc.dram_tensor(name, shape, dt, kind="ExternalInput")

    # ---- inputs
    framesT = din("framesT", [D, T])
    smask = din("smask", [1, T])
    cosB = din("cosB", [128, T])
    sinB = din("sinB", [128, T])
    w_in = din("w_in", [F, D]); b_in = din("b_in", [1, D])
    w_si = din("w_si", [NSSM, D, 2 * D]); b_si = din("b_si", [NSSM, 128, 8], F32)
    cw = din("cw", [NSSM, 128, KC, 3], F32); cb = din("cb", [NSSM, 128, KC], F32)
    w_so = din("w_so", [NSSM, D, D]); b_so = din("b_so", [NSSM, 1, D])
    w_q = din("w_q", [NATTN, D, D]); b_q = din("b_q", [NATTN, 128, KC], F32)
    w_k = din("w_k", [NATTN, D, D]); b_k = din("b_k", [NATTN, 128, KC], F32)
    w_qr = din("w_qr", [NATTN, D, D]); b_qr = din("b_qr", [NATTN, 128, KC], F32)
    w_kr = din("w_kr", [NATTN, D, D]); b_kr = din("b_kr", [NATTN, 128, KC], F32)
    w_v = din("w_v", [NATTN, D, D]); b_v = din("b_v", [NATTN, 1, D])
    w_ao = din("w_ao", [NATTN, D, D]); b_ao = din("b_ao", [NATTN, 1, D])
    w_f1 = din("w_f1", [NATTN, D, FFD]); b_f1 = din("b_f1", [NATTN, 128, FFD // 128], F32)
    w_f2 = din("w_f2", [NATTN, FFD, D]); b_f2 = din("b_f2", [NATTN, 1, D])
    w_fr = din("w_fr", [D, F]); b_fr = din("b_fr", [1, F])
    w_sy = din("w_sy", [D, V]); b_sy = din("b_sy", [1, V])

    out_frame = nc.dram_tensor("out_frame", [T, F], F32, kind="ExternalOutput")
    out_sym = nc.dram_tensor("out_sym", [T, V], F32, kind="ExternalOutput")

    RG = [[0, 1], [2, 3], [4, 5], [6, 7]]

    with tile.TileContext(nc) as tc:
        ctx = contextlib.ExitStack()
        with ctx:
            persist = ctx.enter_context(tc.tile_pool(name="persist", bufs=1))
            wpool = ctx.enter_context(tc.tile_pool(name="wpool", bufs=2))
            act = ctx.enter_context(tc.tile_pool(name="act", bufs=1))
            big = ctx.enter_context(tc.tile_pool(name="big", bufs=1))
            sm2 = ctx.enter_context(tc.tile_pool(name="sm2", bufs=2))
            sm1 = ctx.enter_context(tc.tile_pool(name="sm1", bufs=1))
            wpool1 = ctx.enter_context(tc.tile_pool(name="wpool1", bufs=2))
            sm3 = ctx.enter_context(tc.tile_pool(name="sm3", bufs=3))
            ps_mm = ctx.enter_context(tc.tile_pool(name="ps_mm", bufs=3, space="PSUM"))
            ps_av = ctx.enter_context(tc.tile_pool(name="ps_av", bufs=3, space="PSUM"))
            ps_tr = ctx.enter_context(tc.tile_pool(name="ps_tr", bufs=2, space="PSUM"))
            dram = ctx.enter_context(tc.tile_pool(name="dram", bufs=2, space="DRAM"))

            ident = persist.tile([128, 128], DTM)
            make_identity(nc, ident)
            ones_f32 = persist.tile([1, 64], F32)
            nc.vector.memset(ones_f32[:], 1.0)
            epst = persist.tile([128, 1], F32)
            nc.vector.memset(epst[:], EPS)
            zerob = persist.tile([128, 1], F32)
            nc.vector.memset(zerob[:], 0.0)

            smask_bc = persist.tile([128, T], DTM)
            nc.sync.dma_start(smask_bc[:], smask[:].to_broadcast((128, T)))
            cosB_sb = persist.tile([128, T], DTM)
            nc.sync.dma_start(cosB_sb[:], cosB[:])
            sinB_sb = persist.tile([128, T], DTM)
            nc.sync.dma_start(sinB_sb[:], sinB[:])

            x = persist.tile([128, NT, D], F32)     # residual stream, token-major

            # ---------- helpers ----------
            def load_w(w2d, kchunks, n_out, tag):
                pool_ = wpool1 if tag == "w_d_d2" else wpool
                t_ = pool_.tile([128, kchunks, n_out], DTM, tag=tag)
                nc.sync.dma_start(t_[:], w2d.rearrange("(kc p) n -> p kc n", p=128))
                return t_

            def load_bias_fm(b2d, ncols, tag):
                t_ = wpool.tile([128, ncols], F32, tag=tag)
                nc.sync.dma_start(t_[:], b2d)
                return t_

            def load_bias_bc(b2d, n, tag):
                t_ = wpool.tile([128, n], DTM, tag=tag)
                nc.sync.dma_start(t_[:], b2d.to_broadcast((128, n)))
                return t_

            def mm_fm(out_fm, actT, w_sb, bias_fm, nchunks, epi_func, off=0):
                """out_fm[:, off+n, :] = epi(w-cols.T @ actT-chunks + bias)."""
                for n in range(nchunks):
                    for (qo, qw) in PANELS:
                        ps = ps_mm.tile([128, 512], F32, tag="ps_mm")
                        for kc_ in range(KC):
                            nc.tensor.matmul(
                                ps[:, :qw], lhsT=w_sb[:, kc_, ts(off + n, 128)],
                                rhs=actT[:, kc_, qo:qo + qw],
                                start=(kc_ == 0), stop=(kc_ == KC - 1))
                        nc.scalar.activation(
                            out=out_fm[:, off + n, qo:qo + qw], in_=ps[:, :qw],
                            func=epi_func, bias=bias_fm[:, off + n:off + n + 1])

            def mm_tm(actT, kchunks, w_sb, n_out, consumer):
                """token-major out: per t-tile psum [128, n<=512] -> consumer
                (bias is applied by the consumer, off the PE)."""
                nh = (n_out + 511) // 512
                for t_ in range(NT):
                    for nh_i in range(nh):
                        n0 = nh_i * 512
                        nw = min(512, n_out - n0)
                        ps = ps_mm.tile([128, 512], F32, tag="ps_mm")
                        for kc_ in range(kchunks):
                            nc.tensor.matmul(
                                ps[:, :nw], lhsT=actT[:, kc_, ts(t_, 128)],
                                rhs=w_sb[:, kc_, n0:n0 + nw],
                                start=(kc_ == 0), stop=(kc_ == kchunks - 1))
                        consumer(t_, ps[:, :nw], n0, nw)

            def layernorm_T(xt):
                """Pure-normalize x (token-major fp32) -> transposed [128, KC, T] bf16.

                Fully per-tile pipeline: tile t's transposes (PE) overlap tile
                t+1's stats (DVE) instead of a batched-stats barrier."""
                xhatT = act.tile([128, KC, T], DTM, tag="xhatT", name="xhatT")
                for t_ in range(NT):
                    st6 = sm3.tile([128, 6], F32, tag="ln_st6", name="ln_st6")
                    mv = sm3.tile([128, 2], F32, tag="ln_mv", name="ln_mv")
                    rstd = sm3.tile([128, 1], F32, tag="ln_rstd", name="ln_rstd")
                    nc.vector.bn_stats(st6[:], xt[:, t_, :])
                    nc.vector.bn_aggr(mv[:], st6[:])
                    nc.scalar.activation(out=rstd[:], in_=mv[:, 1:2], func=AF.Sqrt,
                                         bias=epst[:])
                    nc.vector.reciprocal(rstd[:], rstd[:])
                    xh = sm2.tile([128, D], DTM, tag="ln_xh", name="ln_xh")
                    nc.vector.tensor_scalar(
                        out=xh[:], in0=xt[:, t_, :],
                        scalar1=mv[:, 0:1], scalar2=rstd[:, 0:1],
                        op0=ALU.subtract, op1=ALU.mult)
                    pt = ps_tr.tile([128, 512], DTM, tag="ps_tr", name="pt")
                    for kc_ in range(KC):
                        nc.tensor.transpose(pt[:, ts(kc_, 128)],
                                            xh[:, ts(kc_, 128)], ident[:])
                    nc.vector.tensor_copy(
                        xhatT[:, :, ts(t_, 128)],
                        pt[:].rearrange("p (c q) -> p c q", q=128))
                return xhatT

            def rope_combine(qk, qkr):
                """qk <- qk*cosB + qkr*sinB (rotation term computed via folded weights)."""
                for c in range(KC):
                    tmp = sm2.tile([128, T], DTM, tag="ctmp_a", name="rope_tmp")
                    sn = sm2.tile([128, T], DTM, tag="ctmp_b", name="rope_s")
                    nc.vector.tensor_mul(tmp[:], qk[:, c, :], cosB_sb[:])
                    nc.gpsimd.tensor_mul(sn[:], qkr[:, c, :], sinB_sb[:])
                    nc.vector.tensor_add(qk[:, c, :], tmp[:], sn[:])

            # ---------- input projection ----------
            framesT_sb = act.tile([128, KC, T], DTM, tag="xhatT", name="framesT_sb")
            nc.sync.dma_start(framesT_sb[:],
                              framesT[:].rearrange("(kc p) t -> p kc t", p=128))
            w_in_sb = load_w(w_in[:], KC, D, "w_d_d")
            b_in_sb = load_bias_bc(b_in[:], D, "b_row_d")

            def into_x(t_, ps, n0, nw):
                nc.vector.tensor_add(x[:, t_, n0:n0 + nw], ps, b_in_sb[:, n0:n0 + nw])
            mm_tm(framesT_sb, KC, w_in_sb, D, into_x)

            # ---------- layers ----------
            si = ai = 0
            for layer in range(NSSM + NATTN):
                if layer in ATTN_POS:
                    a = ai; ai += 1
                    # === attention block ===
                    xhatT = layernorm_T(x)

                    # K projection first so the exchange starts early
                    w_k_sb = load_w(w_k[a], KC, D, "w_d_d")
                    b_k_sb = load_bias_fm(b_k[a], KC, "b_fm_4")
                    k_fm = act.tile([128, KC, T], DTM, tag="mix2", name="k_fm")
                    mm_fm(k_fm, xhatT, w_k_sb, b_k_sb, KC, AF.Identity)
                    w_kr_sb = load_w(w_kr[a], KC, D, "w_d_d")
                    b_kr_sb = load_bias_fm(b_kr[a], KC, "b_fm_4")
                    kr_fm = act.tile([128, KC, T], DTM, tag="rot", name="kr_fm")
                    mm_fm(kr_fm, xhatT, w_kr_sb, b_kr_sb, KC, AF.Identity)
                    rope_combine(k_fm, kr_fm)
                    k_bin = dram.tile([D, 1024], DTM, tag="k_bin", name="k_bin")
                    nc.sync.dma_start(
                        k_bin[:].rearrange("(c p) t -> p c t", p=128),
                        k_fm[:, :, HALO:HALO + 1024])
                    k_bout = dram.tile([2 * D, 1024], DTM, tag="k_bout", name="k_bout")
                    nc.gpsimd.collective_compute(
                        "AllGather", ALU.bypass, replica_groups=RG,
                        ins=[k_bin.opt()], outs=[k_bout.opt()])
                    k_full = big.tile([128, KC, S], DTM, tag="k_full", name="k_full")
                    nc.sync.dma_start(
                        k_full[:, :, 0:1024],
                        k_bout[0:D].rearrange("(c p) t -> p c t", p=128))
                    nc.sync.dma_start(
                        k_full[:, :, 1024:2048],
                        k_bout[D:2 * D].rearrange("(c p) t -> p c t", p=128))

                    # V projection (token-major) -> exchange
                    w_v_sb = load_w(w_v[a], KC, D, "w_d_d")
                    b_v_sb = load_bias_bc(b_v[a], D, "b_row_d")
                    v_loc = act.tile([128, NT, D], DTM, tag="mix1", name="v_loc")

                    def into_v(t_, ps, n0, nw):
                        nc.vector.tensor_add(v_loc[:, t_, n0:n0 + nw], ps,
                                             b_v_sb[:, n0:n0 + nw])
                    mm_tm(xhatT, KC, w_v_sb, D, into_v)
                    # exchange V in two feature halves so heads 0-3 can
                    # start their PV matmuls after only the first collective
                    v_tm = big.tile([128, NKV, H, HD + 1], DTM, tag="v_tm", name="v_tm")
                    for vh in range(2):
                        c0 = vh * 256
                        v_bin = dram.tile([1024, 256], DTM, tag="v_bin", name="v_bin")
                        nc.sync.dma_start(v_bin[0:64], v_loc[64:128, 0, c0:c0 + 256])
                        nc.sync.dma_start(
                            v_bin[64:960].rearrange("(c p) n -> p c n", p=128),
                            v_loc[:, 1:8, c0:c0 + 256])
                        nc.sync.dma_start(v_bin[960:1024], v_loc[0:64, 8, c0:c0 + 256])
                        v_bout = dram.tile([S, 256], DTM, tag="v_bout", name="v_bout")
                        nc.gpsimd.collective_compute(
                            "AllGather", ALU.bypass, replica_groups=RG,
                            ins=[v_bin.opt()], outs=[v_bout.opt()])
                        for h_ in range(4):
                            nc.sync.dma_start(
                                v_tm[:, :, vh * 4 + h_, 0:HD],
                                v_bout[:, h_ * HD:(h_ + 1) * HD].rearrange(
                                    "(kc p) d -> p kc d", p=128))
                    nc.vector.memset(v_tm[:, :, :, HD:HD + 1], 1.0)

                    # Q projection (overlaps the collectives)
                    w_q_sb = load_w(w_q[a], KC, D, "w_d_d")
                    b_q_sb = load_bias_fm(b_q[a], KC, "b_fm_4")
                    q_fm = act.tile([128, KC, T], DTM, tag="q_fm", name="q_fm")
                    mm_fm(q_fm, xhatT, w_q_sb, b_q_sb, KC, AF.Identity)
                    w_qr_sb = load_w(w_qr[a], KC, D, "w_d_d")
                    b_qr_sb = load_bias_fm(b_qr[a], KC, "b_fm_4")
                    qr_fm = act.tile([128, KC, T], DTM, tag="rot", name="qr_fm")
                    mm_fm(qr_fm, xhatT, w_qr_sb, b_qr_sb, KC, AF.Identity)
                    rope_combine(q_fm, qr_fm)

                    # attention core: q restricted to the columns whose
                    # outputs are still needed (local + remaining conv halo)
                    ctx_fm = act.tile([128, KC, T], DTM, tag="mix1", name="ctx_fm")
                    nc.vector.memset(ctx_fm[:], 0.0)
                    for h in range(H):
                        r0 = (h % 2) * 64
                        ch = h // 2
                        av_tiles = [ps_av.tile([65, 512], F32, tag="ps_av", name="av")
                                    for _ in PANELS]
                        for c in range(NKV):
                            ex = sm3.tile([128, T], DTM, tag="ebuf", name="ex")
                            for j, (qo, qw) in enumerate(QPANELS):
                                ps = ps_mm.tile([128, 512], F32, tag="ps_mm", name="sc")
                                nc.tensor.matmul(
                                    ps[:, :qw],
                                    lhsT=k_full[r0:r0 + 64, ch, ts(c, 128)],
                                    rhs=q_fm[r0:r0 + 64, ch, qo:qo + qw],
                                    start=True, stop=True)
                                nc.scalar.activation(
                                    out=ex[:, qo:qo + qw], in_=ps[:, :qw],
                                    func=AF.Exp, scale=0.125)
                            for j, (qo, qw) in enumerate(QPANELS):
                                nc.tensor.matmul(
                                    av_tiles[j][:, :qw],
                                    lhsT=v_tm[:, c, h, :],
                                    rhs=ex[:, qo:qo + qw],
                                    start=(c == 0), stop=(c == NKV - 1))
                        inv = sm1.tile([1, T], F32, tag="attn_inv", name="inv")
                        invb = sm1.tile([64, T], F32, tag="attn_invb", name="invb")
                        for j, (qo, qw) in enumerate(QPANELS):
                            nc.vector.reciprocal(inv[:, qo:qo + qw],
                                                 av_tiles[j][64:65, :qw])
                            ib_ps = ps_tr.tile([64, 512], F32, tag="ps_tr", name="ib")
                            nc.tensor.matmul(ib_ps[:, :qw],
                                             lhsT=ones_f32[:],
                                             rhs=inv[:, qo:qo + qw],
                                             start=True, stop=True)
                            nc.vector.tensor_copy(invb[:, qo:qo + qw], ib_ps[:, :qw])
                        if r0 == 0:
                            for j, (qo, qw) in enumerate(QPANELS):
                                nc.vector.tensor_mul(
                                    ctx_fm[0:64, ch, qo:qo + qw],
                                    av_tiles[j][0:64, :qw], invb[:, qo:qo + qw])
                        else:
                            stage = sm2.tile([64, T], DTM, tag="ctx_stage", name="cst")
                            for j, (qo, qw) in enumerate(QPANELS):
                                nc.vector.tensor_mul(
                                    stage[:, qo:qo + qw],
                                    av_tiles[j][0:64, :qw], invb[:, qo:qo + qw])
                            nc.sync.dma_start(ctx_fm[64:128, ch, :], stage[:])

                    # output projection + residual
                    w_ao_sb = load_w(w_ao[a], KC, D, "w_d_d")
                    b_ao_sb = load_bias_bc(b_ao[a], D, "b_row_d")

                    def into_res(t_, ps, n0, nw):
                        nc.vector.tensor_add(x[:, t_, n0:n0 + nw],
                                             x[:, t_, n0:n0 + nw], ps)
                        nc.gpsimd.tensor_add(x[:, t_, n0:n0 + nw],
                                             x[:, t_, n0:n0 + nw],
                                             b_ao_sb[:, n0:n0 + nw])
                    mm_tm(ctx_fm, KC, w_ao_sb, D, into_res)

                    # === FF block (quarters over the 2048 hidden dim,
                    # each quarter's f2 partial sum added straight into x) ===
                    xhatT = layernorm_T(x)
                    for hf in range(4):
                        w_f1_sb = wpool.tile([128, KC, 512], DTM, tag="w_d_d",
                                             name="w_f1_sb")
                        nc.sync.dma_start(
                            w_f1_sb[:],
                            w_f1[a][:, hf * 512:(hf + 1) * 512].rearrange(
                                "(kc p) n -> p kc n", p=128))
                        b_f1_sb = wpool.tile([128, 4], F32, tag="b_fm_4", name="b_f1_sb")
                        nc.sync.dma_start(b_f1_sb[:], b_f1[a][:, hf * 4:(hf + 1) * 4])
                        g_fm = act.tile([128, 4, T], DTM, tag="mix1", name="g_fm")
                        mm_fm(g_fm, xhatT, w_f1_sb, b_f1_sb, 4, AF.Gelu)
                        w_f2_sb = wpool.tile([128, 4, D], DTM, tag="w_d_d",
                                             name="w_f2_sb")
                        nc.sync.dma_start(
                            w_f2_sb[:],
                            w_f2[a][hf * 512:(hf + 1) * 512].rearrange(
                                "(kc p) n -> p kc n", p=128))
                        if hf == 0:
                            b_f2_bc = load_bias_bc(b_f2[a], D, "b_row_d")

                            def into_ffres(t_, ps, n0, nw):
                                nc.vector.tensor_add(x[:, t_, n0:n0 + nw],
                                                     x[:, t_, n0:n0 + nw], ps)
                                nc.gpsimd.tensor_add(x[:, t_, n0:n0 + nw],
                                                     x[:, t_, n0:n0 + nw],
                                                     b_f2_bc[:, n0:n0 + nw])
                        else:
                            def into_ffres(t_, ps, n0, nw):
                                nc.vector.tensor_add(x[:, t_, n0:n0 + nw],
                                                     x[:, t_, n0:n0 + nw], ps)
                        mm_tm(g_fm, KC, w_f2_sb, D, into_ffres)
                else:
                    i = si; si += 1
                    # === SSM block ===
                    xhatT = layernorm_T(x)
                    w_si_sb = load_w(w_si[i], KC, 2 * D, "w_d_d2")
                    b_si_sb = load_bias_fm(b_si[i], 8, "b_fm_8")
                    z_fm = act.tile([128, 8, T], DTM, tag="g_fm", name="z_fm")
                    mm_fm(z_fm, xhatT, w_si_sb, b_si_sb, 4, AF.Sigmoid, off=0)
                    # gating + seq mask; cand buffer has a zero column each side
                    cand = act.tile([128, KC, T + 2], DTM, tag="mix2", name="cand")
                    nc.vector.memset(cand[:, :, 0:1], 0.0)
                    nc.vector.memset(cand[:, :, T + 1:T + 2], 0.0)
                    cw_sb = wpool.tile([128, KC, 3], F32, tag="cw", name="cw_sb")
                    nc.sync.dma_start(cw_sb[:], cw[i])
                    cb_sb = wpool.tile([128, KC], F32, tag="cb", name="cb_sb")
                    nc.sync.dma_start(cb_sb[:], cb[i])
                    conv_fm = act.tile([128, KC, T], DTM, tag="mix1", name="conv_fm")
                    # cand-chunk projection with gating fused into the PSUM
                    # epilogue: cand = ((z_cand + bias) * sigmoid(z_gate)) * mask
                    for n_ in range(KC):
                        for (qo, qw) in PANELS:
                            ps = ps_mm.tile([128, 512], F32, tag="ps_mm")
                            for kc_ in range(KC):
                                nc.tensor.matmul(
                                    ps[:, :qw], lhsT=w_si_sb[:, kc_, ts(KC + n_, 128)],
                                    rhs=xhatT[:, kc_, qo:qo + qw],
                                    start=(kc_ == 0), stop=(kc_ == KC - 1))
                            nc.vector.scalar_tensor_tensor(
                                out=cand[:, n_, 1 + qo:1 + qo + qw], in0=ps[:, :qw],
                                scalar=b_si_sb[:, KC + n_:KC + n_ + 1],
                                in1=z_fm[:, n_, qo:qo + qw],
                                op0=ALU.add, op1=ALU.mult)
                            nc.gpsimd.tensor_mul(cand[:, n_, 1 + qo:1 + qo + qw],
                                                 cand[:, n_, 1 + qo:1 + qo + qw],
                                                 smask_bc[:, qo:qo + qw])
                    for (o0, o1) in ((0, 640), (640, T)):
                        wd = o1 - o0
                        for c in range(KC):
                            t1 = sm2.tile([128, T], DTM, tag="ctmp_a", name="conv_t1")
                            t2 = sm2.tile([128, T], DTM, tag="ctmp_b", name="conv_t2")
                            nc.vector.tensor_scalar_mul(t1[:, :wd],
                                                        cand[:, c, o0:o1],
                                                        cw_sb[:, c, 0:1])
                            nc.vector.scalar_tensor_tensor(
                                out=t2[:, :wd], in0=cand[:, c, o0 + 1:o1 + 1],
                                scalar=cw_sb[:, c, 1:2], in1=t1[:, :wd],
                                op0=ALU.mult, op1=ALU.add)
                            nc.vector.scalar_tensor_tensor(
                                out=t1[:, :wd], in0=cand[:, c, o0 + 2:o1 + 2],
                                scalar=cw_sb[:, c, 2:3], in1=t2[:, :wd],
                                op0=ALU.mult, op1=ALU.add)
                            nc.scalar.activation(
                                out=conv_fm[:, c, o0:o1], in_=t1[:, :wd],
                                func=AF.Relu, bias=cb_sb[:, c:c + 1])
                    w_so_sb = load_w(w_so[i], KC, D, "w_d_d")
                    b_so_sb = load_bias_bc(b_so[i], D, "b_row_d")

                    def into_res_s(t_, ps, n0, nw):
                        nc.vector.tensor_add(x[:, t_, n0:n0 + nw],
                                             x[:, t_, n0:n0 + nw], ps)
                        nc.gpsimd.tensor_add(x[:, t_, n0:n0 + nw],
                                             x[:, t_, n0:n0 + nw],
                                             b_so_sb[:, n0:n0 + nw])
                    mm_tm(conv_fm, KC, w_so_sb, D, into_res_s)

            # ---------- heads ----------
            hT = layernorm_T(x)
            w_fr_sb = load_w(w_fr[:], KC, F, "w_d_d")
            b_fr_sb = load_bias_bc(b_fr[:], F, "b_row_d")

            def into_frame(t_, ps, n0, nw):
                st_ = sm2.tile([128, 512], F32, tag="ln_xh", name="ost")
                nc.vector.tensor_add(st_[:, :nw], ps, b_fr_sb[:, n0:n0 + nw])
                nc.sync.dma_start(out_frame[ts(t_, 128), n0:n0 + nw], st_[:, :nw])
            mm_tm(hT, KC, w_fr_sb, F, into_frame)

            w_sy_sb = wpool1.tile([128, KC, V], DTM, tag="w_d_d2", name="w_sy_sb")
            nc.sync.dma_start(w_sy_sb[:],
                              w_sy[:].rearrange("(kc p) n -> p kc n", p=128))
            b_sy_sb = wpool1.tile([128, V], DTM, tag="b_row_2d", name="b_sy_sb")
            nc.sync.dma_start(b_sy_sb[:], b_sy[:].to_broadcast((128, V)))

            def into_sym(t_, ps, n0, nw):
                st_ = sm2.tile([128, 512], F32, tag="ln_xh", name="ost")
                nc.vector.tensor_add(st_[:, :nw], ps, b_sy_sb[:, n0:n0 + nw])
                nc.sync.dma_start(out_sym[ts(t_, 128), n0:n0 + nw], st_[:, :nw])
            mm_tm(hT, KC, w_sy_sb, V, into_sym)

    nc.finalize()
    return nc


# ------------------------------------------------------------- host prep

def _host_prep(inputs, np_mm):
    g = {k: np.asarray(v, np.float32) if np.asarray(v).dtype != np.bool_
         else np.asarray(v) for k, v in inputs.items()}

    def fm_bias(b):      # [N] -> [128, N//128]
        return np.ascontiguousarray(b.reshape(-1, 128).T).astype(np.float32)

    P = {}
    P["w_in"] = g["input_w"].astype(np_mm)
    P["b_in"] = g["input_b"][None, :].astype(np_mm)
    wsi = np.empty((NSSM, D, 2 * D), np_mm); bsi = np.empty((NSSM, 128, 8), np.float32)
    cwl = np.empty((NSSM, 128, KC, 3), np.float32)
    cbl = np.empty((NSSM, 128, KC), np.float32)
    wso = np.empty((NSSM, D, D), np_mm); bso = np.empty((NSSM, 1, D), np_mm)
    for i in range(NSSM):
        nw, nb = g["ssm_norm_w"][i], g["ssm_norm_b"][i]
        wsi[i] = (nw[:, None] * g["ssm_in_w"][i]).astype(np_mm)
        bsi[i] = fm_bias(nb @ g["ssm_in_w"][i] + g["ssm_in_b"][i])
        cwl[i] = g["ssm_conv_w"][i].reshape(KC, 128, 3).transpose(1, 0, 2)
        cbl[i] = g["ssm_conv_b"][i].reshape(KC, 128).T
        wso[i] = g["ssm_out_w"][i].astype(np_mm)
        bso[i] = g["ssm_out_b"][i][None, :].astype(np_mm)
    P.update(w_si=wsi, b_si=bsi, cw=cwl, cb=cbl, w_so=wso, b_so=bso)

    def rot_feat(w):
        """Apply rotate_half permutation (with sign) to output features of w [.., D]."""
        wh = w.reshape(w.shape[:-1] + (H, HD))
        out = np.empty_like(wh)
        out[..., 0:32] = -wh[..., 32:64]
        out[..., 32:64] = wh[..., 0:32]
        return out.reshape(w.shape)

    wq = np.empty((NATTN, D, D), np_mm); bq = np.empty((NATTN, 128, KC), np.float32)
    wk = np.empty((NATTN, D, D), np_mm); bk = np.empty((NATTN, 128, KC), np.float32)
    wqr = np.empty((NATTN, D, D), np_mm); bqr = np.empty((NATTN, 128, KC), np.float32)
    wkr = np.empty((NATTN, D, D), np_mm); bkr = np.empty((NATTN, 128, KC), np.float32)
    wv = np.empty((NATTN, D, D), np_mm); bv = np.empty((NATTN, 1, D), np_mm)
    wao = np.empty((NATTN, D, D), np_mm); bao = np.empty((NATTN, 1, D), np_mm)
    wf1 = np.empty((NATTN, D, FFD), np_mm)
    bf1 = np.empty((NATTN, 128, FFD // 128), np.float32)
    wf2 = np.empty((NATTN, FFD, D), np_mm); bf2 = np.empty((NATTN, 1, D), np_mm)
    for a in range(NATTN):
        n1w, n1b = g["a_ln1_w"][a], g["a_ln1_b"][a]
        wqf = n1w[:, None] * g["a_q_w"][a]
        bqf = n1b @ g["a_q_w"][a] + g["a_q_b"][a]
        wkf = n1w[:, None] * g["a_k_w"][a]
        bkf = n1b @ g["a_k_w"][a] + g["a_k_b"][a]
        wq[a] = wqf.astype(np_mm); bq[a] = fm_bias(bqf)
        wk[a] = wkf.astype(np_mm); bk[a] = fm_bias(bkf)
        wqr[a] = rot_feat(wqf).astype(np_mm); bqr[a] = fm_bias(rot_feat(bqf))
        wkr[a] = rot_feat(wkf).astype(np_mm); bkr[a] = fm_bias(rot_feat(bkf))
        wv[a] = (n1w[:, None] * g["a_v_w"][a]).astype(np_mm)
        bv[a] = (n1b @ g["a_v_w"][a] + g["a_v_b"][a])[None, :].astype(np_mm)
        wao[a] = g["a_o_w"][a].astype(np_mm)
        bao[a] = g["a_o_b"][a][None, :].astype(np_mm)
        n2w, n2b = g["a_ln2_w"][a], g["a_ln2_b"][a]
        wf1[a] = (n2w[:, None] * g["a_f1_w"][a]).astype(np_mm)
        bf1[a] = fm_bias(n2b @ g["a_f1_w"][a] + g["a_f1_b"][a])
        wf2[a] = g["a_f2_w"][a].astype(np_mm)
        bf2[a] = g["a_f2_b"][a][None, :].astype(np_mm)
    P.update(w_q=wq, b_q=bq, w_k=wk, b_k=bk, w_qr=wqr, b_qr=bqr,
             w_kr=wkr, b_kr=bkr, w_v=wv, b_v=bv, w_ao=wao, b_ao=bao,
             w_f1=wf1, b_f1=bf1, w_f2=wf2, b_f2=bf2)

    fnw, fnb = g["fn_w"], g["fn_b"]
    P["w_fr"] = (fnw[:, None] * g["frame_w"]).astype(np_mm)
    P["b_fr"] = (fnb @ g["frame_w"] + g["frame_b"])[None, :].astype(np_mm)
    P["w_sy"] = (fnw[:, None] * g["sym_w"]).astype(np_mm)
    P["b_sy"] = (fnb @ g["sym_w"] + g["sym_b"])[None, :].astype(np_mm)

    frames = g["frames"]
    inv_freq = 1.0 / (10000.0 ** (np.arange(0, HD, 2, dtype=np.float32) / HD))
    per_core = []
    for c in range(8):
        b, hhalf = c // 2, c % 2
        start = hhalf * 1024 - HALO
        idx = np.arange(start, start + T)
        inseq = (idx >= 0) & (idx < S)
        fr = np.zeros((T, F), np.float32)
        fr[inseq] = frames[b][idx[inseq]]
        freqs = np.outer(idx.astype(np.float32), inv_freq)
        cos32 = np.cos(freqs).astype(np.float32)   # [T, 32]
        sin32 = np.sin(freqs).astype(np.float32)
        pr = np.arange(128) % 32
        d = dict(P)
        d["framesT"] = np.ascontiguousarray(fr.T).astype(np_mm)
        d["smask"] = inseq.astype(np.float32)[None, :].astype(np_mm)
        d["cosB"] = np.ascontiguousarray(cos32[:, pr].T).astype(np_mm)
        d["sinB"] = np.ascontiguousarray(sin32[:, pr].T).astype(np_mm)
        per_core.append(d)
    return per_core


# ----------------------------------------------------------------- entry

def kernel(**inputs):
    dt_mm = os.environ.get("MJM_DT", "bfloat16")
    if dt_mm == "bfloat16":
        import ml_dtypes
        np_mm = ml_dtypes.bfloat16
    else:
        np_mm = np.float32

    trace = os.environ.get("MJM_TRACE", "0") == "1"
    if trace:
        import sys, types
        if "antenv.axon_hooks" not in sys.modules:
            try:
                from trn_agent_boot.trn_boot import _ntff_profile_via_ctypes
                hook = _ntff_profile_via_ctypes("/opt/axon/libaxon_pjrt.so")
                mod = types.ModuleType("antenv.axon_hooks")
                mod.get_axon_ntff_profile_hook = lambda: hook
                sys.modules["antenv.axon_hooks"] = mod
            except Exception:
                trace = False

    from concourse.bass_utils import run_bass_kernel_spmd

    key = ("nc", dt_mm)
    if key not in _CACHE:
        _CACHE[key] = _build_nc(dt_mm)
    nc = _CACHE[key]

    in_maps = _host_prep(inputs, np_mm)
    last_err = None
    res = None
    for _attempt in range(3):
        try:
            res = run_bass_kernel_spmd(nc, in_maps, core_ids=list(range(8)),
                                       trace=trace)
            break
        except Exception as e:                       # transient NRT/worker errors
            last_err = e
            import time as _time
            _time.sleep(5)
    if res is None:
        raise last_err

    if trace and res.exec_time_ns is not None:
        print(f"HW exec time: {res.exec_time_ns} ns")

    frame_out = np.zeros((B, S, F), np.float32)
    sym_out = np.zeros((B, S, V), np.float32)
    for c in range(8):
        b, hhalf = c // 2, c % 2
        sl = slice(hhalf * 1024, hhalf * 1024 + 1024)
        frame_out[b, sl] = res.results[c]["out_frame"][HALO:HALO + 1024]
        sym_out[b, sl] = res.results[c]["out_sym"][HALO:HALO + 1024]
    return frame_out, sym_out


# revision 32
# speedup vs baseline: 1.0691x; 1.0169x over previous
"""Trainium2 Bass kernel for MiniJMamba (12-layer SSM+attention hybrid).

Sharding: 8 cores = batch(4) x seq-half(2). Each core processes a
1152-token window (1024 local + 64-token halo each side, zero-padded at
sequence ends). The 10 depthwise convs each consume one token of halo
validity (10 <= 64). At the two attention layers, cores exchange K/V
halves with their pair core via AllGather over groups [2b, 2b+1].

Host-side prep folds every LayerNorm affine into the following matmul
weights, so on-device LN is pure normalization. Activations use two
layouts: token-major [128 tok, ...] for LN and feature-major [feat, tok]
as matmul lhsT; PE transposes bridge them. Matmul operands are bf16
(PSUM accumulation fp32); the residual stream and LN stats stay fp32.
"""

import os
import numpy as np

B, S, F, D, H, V = 4, 2048, 512, 512, 8, 1024
HD, FFD = 64, 2048
NSSM, NATTN = 10, 2
ATTN_POS = (4, 8)
EPS = 1e-5
HALO = 64
T = 1024 + 2 * HALO            # 1152 tokens per core window
NT = T // 128                  # 9 token tiles
KC = D // 128                  # 4 feature chunks
PANELS = [(0, 512), (512, 512), (1024, T - 1024)]  # free-dim panels of T
# attention query window: local tokens + 6-token halo each side (enough for
# the convolutions remaining after each attention layer)
QPANELS = [(58, 512), (570, 512), (1082, 12)]
NKV = S // 128                 # 16 key chunks (full sequence)

_CACHE = {}


# ----------------------------------------------------------------- build

def _build_nc(dt_mm_name="bfloat16"):
    import concourse.bass as bass
    import concourse.tile as tile
    from concourse import bacc, mybir
    from concourse.masks import make_identity
    import contextlib

    F32 = mybir.dt.float32
    DTM = getattr(mybir.dt, dt_mm_name)
    AF = mybir.ActivationFunctionType
    ALU = mybir.AluOpType
    ts = bass.ts

    nc = bacc.Bacc(None, target_bir_lowering=False, num_devices=8)

    def din(name, shape, dt=DTM):
        return nc.dram_tensor(name, shape, dt, kind="ExternalInput")

    # ---- inputs
    framesT = din("framesT", [D, T])
    smask = din("smask", [1, T])
    cosB = din("cosB", [128, T])
    sinB = din("sinB", [128, T])
    w_in = din("w_in", [F, D]); b_in = din("b_in", [1, D])
    w_si = din("w_si", [NSSM, D, 2 * D]); b_si = din("b_si", [NSSM, 128, 8], F32)
    cw = din("cw", [NSSM, 128, KC, 3], F32); cb = din("cb", [NSSM, 128, KC], F32)
    w_so = din("w_so", [NSSM, D, D]); b_so = din("b_so", [NSSM, 1, D])
    w_q = din("w_q", [NATTN, D, D]); b_q = din("b_q", [NATTN, 128, KC], F32)
    w_k = din("w_k", [NATTN, D, D]); b_k = din("b_k", [NATTN, 128, KC], F32)
    w_qr = din("w_qr", [NATTN, D, D]); b_qr = din("b_qr", [NATTN, 128, KC], F32)
    w_kr = din("w_kr", [NATTN, D, D]); b_kr = din("b_kr", [NATTN, 128, KC], F32)
    w_v = din("w_v", [NATTN, D, D]); b_v = din("b_v", [NATTN, 1, D])
    w_ao = din("w_ao", [NATTN, D, D]); b_ao = din("b_ao", [NATTN, 1, D])
    w_f1 = din("w_f1", [NATTN, D, FFD]); b_f1 = din("b_f1", [NATTN, 128, FFD // 128], F32)
    w_f2 = din("w_f2", [NATTN, FFD, D]); b_f2 = din("b_f2", [NATTN, 1, D])
    w_fr = din("w_fr", [D, F]); b_fr = din("b_fr", [1, F])
    w_sy = din("w_sy", [D, V]); b_sy = din("b_sy", [1, V])

    out_frame = nc.dram_tensor("out_frame", [T, F], F32, kind="ExternalOutput")
    out_sym = nc.dram_tensor("out_sym", [T, V], F32, kind="ExternalOutput")

    RG = [[0, 1], [2, 3], [4, 5], [6, 7]]

    with tile.TileContext(nc) as tc:
        ctx = contextlib.ExitStack()
        with ctx:
            persist = ctx.enter_context(tc.tile_pool(name="persist", bufs=1))
            wpool = ctx.enter_context(tc.tile_pool(name="wpool", bufs=3))
            act = ctx.enter_context(tc.tile_pool(name="act", bufs=1))
            big = ctx.enter_context(tc.tile_pool(name="big", bufs=1))
            sm2 = ctx.enter_context(tc.tile_pool(name="sm2", bufs=2))
            sm1 = ctx.enter_context(tc.tile_pool(name="sm1", bufs=1))
            wpool1 = ctx.enter_context(tc.tile_pool(name="wpool1", bufs=2))
            sm3 = ctx.enter_context(tc.tile_pool(name="sm3", bufs=4))
            ps_mm = ctx.enter_context(tc.tile_pool(name="ps_mm", bufs=3, space="PSUM"))
            ps_av = ctx.enter_context(tc.tile_pool(name="ps_av", bufs=3, space="PSUM"))
            ps_tr = ctx.enter_context(tc.tile_pool(name="ps_tr", bufs=2, space="PSUM"))
            dram = ctx.enter_context(tc.tile_pool(name="dram", bufs=2, space="DRAM"))

            ident = persist.tile([128, 128], DTM)
            make_identity(nc, ident)
            ones_f32 = persist.tile([1, 64], F32)
            nc.vector.memset(ones_f32[:], 1.0)
            epst = persist.tile([128, 1], F32)
            nc.vector.memset(epst[:], EPS)
            zerob = persist.tile([128, 1], F32)
            nc.vector.memset(zerob[:], 0.0)

            smask_bc = persist.tile([128, T], DTM)
            nc.sync.dma_start(smask_bc[:], smask[:].to_broadcast((128, T)))
            cosB_sb = persist.tile([128, T], DTM)
            nc.sync.dma_start(cosB_sb[:], cosB[:])
            sinB_sb = persist.tile([128, T], DTM)
            nc.sync.dma_start(sinB_sb[:], sinB[:])

            x = persist.tile([128, NT, D], F32)     # residual stream, token-major

            # ---------- helpers ----------
            def load_w(w2d, kchunks, n_out, tag):
                pool_ = wpool1 if tag == "w_d_d2" else wpool
                t_ = pool_.tile([128, kchunks, n_out], DTM, tag=tag)
                nc.sync.dma_start(t_[:], w2d.rearrange("(kc p) n -> p kc n", p=128))
                return t_

            def load_bias_fm(b2d, ncols, tag):
                t_ = wpool.tile([128, ncols], F32, tag=tag)
                nc.sync.dma_start(t_[:], b2d)
                return t_

            def load_bias_bc(b2d, n, tag):
                t_ = wpool.tile([128, n], DTM, tag=tag)
                nc.sync.dma_start(t_[:], b2d.to_broadcast((128, n)))
                return t_

            def mm_fm(out_fm, actT, w_sb, bias_fm, nchunks, epi_func, off=0):
                """out_fm[:, off+n, :] = epi(w-cols.T @ actT-chunks + bias)."""
                for n in range(nchunks):
                    for (qo, qw) in PANELS:
                        ps = ps_mm.tile([128, 512], F32, tag="ps_mm")
                        for kc_ in range(KC):
                            nc.tensor.matmul(
                                ps[:, :qw], lhsT=w_sb[:, kc_, ts(off + n, 128)],
                                rhs=actT[:, kc_, qo:qo + qw],
                                start=(kc_ == 0), stop=(kc_ == KC - 1))
                        nc.scalar.activation(
                            out=out_fm[:, off + n, qo:qo + qw], in_=ps[:, :qw],
                            func=epi_func, bias=bias_fm[:, off + n:off + n + 1])

            def mm_tm(actT, kchunks, w_sb, n_out, consumer):
                """token-major out: per t-tile psum [128, n<=512] -> consumer
                (bias is applied by the consumer, off the PE)."""
                nh = (n_out + 511) // 512
                for t_ in range(NT):
                    for nh_i in range(nh):
                        n0 = nh_i * 512
                        nw = min(512, n_out - n0)
                        ps = ps_mm.tile([128, 512], F32, tag="ps_mm")
                        for kc_ in range(kchunks):
                            nc.tensor.matmul(
                                ps[:, :nw], lhsT=actT[:, kc_, ts(t_, 128)],
                                rhs=w_sb[:, kc_, n0:n0 + nw],
                                start=(kc_ == 0), stop=(kc_ == kchunks - 1))
                        consumer(t_, ps[:, :nw], n0, nw)

            def layernorm_T(xt):
                """Pure-normalize x (token-major fp32) -> transposed [128, KC, T] bf16.

                Fully per-tile pipeline: tile t's transposes (PE) overlap tile
                t+1's stats (DVE) instead of a batched-stats barrier."""
                xhatT = act.tile([128, KC, T], DTM, tag="xhatT", name="xhatT")
                for t_ in range(NT):
                    st6 = sm3.tile([128, 6], F32, tag="ln_st6", name="ln_st6")
                    mv = sm3.tile([128, 2], F32, tag="ln_mv", name="ln_mv")
                    rstd = sm3.tile([128, 1], F32, tag="ln_rstd", name="ln_rstd")
                    nc.vector.bn_stats(st6[:], xt[:, t_, :])
                    nc.vector.bn_aggr(mv[:], st6[:])
                    nc.scalar.activation(out=rstd[:], in_=mv[:, 1:2], func=AF.Sqrt,
                                         bias=epst[:])
                    nc.vector.reciprocal(rstd[:], rstd[:])
                    xh = sm2.tile([128, D], DTM, tag="ln_xh", name="ln_xh")
                    nc.vector.tensor_scalar(
                        out=xh[:], in0=xt[:, t_, :],
                        scalar1=mv[:, 0:1], scalar2=rstd[:, 0:1],
                        op0=ALU.subtract, op1=ALU.mult)
                    pt = ps_tr.tile([128, 512], DTM, tag="ps_tr", name="pt")
                    for kc_ in range(KC):
                        nc.tensor.transpose(pt[:, ts(kc_, 128)],
                                            xh[:, ts(kc_, 128)], ident[:])
                    nc.vector.tensor_copy(
                        xhatT[:, :, ts(t_, 128)],
                        pt[:].rearrange("p (c q) -> p c q", q=128))
                return xhatT

            def rope_combine(qk, qkr):
                """qk <- qk*cosB + qkr*sinB (rotation term computed via folded weights)."""
                for c in range(KC):
                    tmp = sm2.tile([128, T], DTM, tag="ctmp_a", name="rope_tmp")
                    sn = sm2.tile([128, T], DTM, tag="ctmp_b", name="rope_s")
                    nc.vector.tensor_mul(tmp[:], qk[:, c, :], cosB_sb[:])
                    nc.gpsimd.tensor_mul(sn[:], qkr[:, c, :], sinB_sb[:])
                    nc.vector.tensor_add(qk[:, c, :], tmp[:], sn[:])

            # ---------- input projection ----------
            framesT_sb = act.tile([128, KC, T], DTM, tag="xhatT", name="framesT_sb")
            nc.sync.dma_start(framesT_sb[:],
                              framesT[:].rearrange("(kc p) t -> p kc t", p=128))
            w_in_sb = load_w(w_in[:], KC, D, "w_d_d")
            b_in_sb = load_bias_bc(b_in[:], D, "b_row_d")

            def into_x(t_, ps, n0, nw):
                nc.vector.tensor_add(x[:, t_, n0:n0 + nw], ps, b_in_sb[:, n0:n0 + nw])
            mm_tm(framesT_sb, KC, w_in_sb, D, into_x)

            # ---------- layers ----------
            si = ai = 0
            for layer in range(NSSM + NATTN):
                if layer in ATTN_POS:
                    a = ai; ai += 1
                    # === attention block ===
                    xhatT = layernorm_T(x)

                    # K projection first so the exchange starts early
                    w_k_sb = load_w(w_k[a], KC, D, "w_d_d")
                    b_k_sb = load_bias_fm(b_k[a], KC, "b_fm_4")
                    k_fm = act.tile([128, KC, T], DTM, tag="mix2", name="k_fm")
                    mm_fm(k_fm, xhatT, w_k_sb, b_k_sb, KC, AF.Identity)
                    w_kr_sb = load_w(w_kr[a], KC, D, "w_d_d")
                    b_kr_sb = load_bias_fm(b_kr[a], KC, "b_fm_4")
                    kr_fm = act.tile([128, KC, T], DTM, tag="rot", name="kr_fm")
                    mm_fm(kr_fm, xhatT, w_kr_sb, b_kr_sb, KC, AF.Identity)
                    rope_combine(k_fm, kr_fm)
                    k_bin = dram.tile([D, 1024], DTM, tag="k_bin", name="k_bin")
                    nc.sync.dma_start(
                        k_bin[:].rearrange("(c p) t -> p c t", p=128),
                        k_fm[:, :, HALO:HALO + 1024])
                    k_bout = dram.tile([2 * D, 1024], DTM, tag="k_bout", name="k_bout")
                    nc.gpsimd.collective_compute(
                        "AllGather", ALU.bypass, replica_groups=RG,
                        ins=[k_bin.opt()], outs=[k_bout.opt()])
                    k_full = big.tile([128, KC, S], DTM, tag="k_full", name="k_full")
                    nc.sync.dma_start(
                        k_full[:, :, 0:1024],
                        k_bout[0:D].rearrange("(c p) t -> p c t", p=128))
                    nc.sync.dma_start(
                        k_full[:, :, 1024:2048],
                        k_bout[D:2 * D].rearrange("(c p) t -> p c t", p=128))

                    # V projection (token-major) -> exchange
                    w_v_sb = load_w(w_v[a], KC, D, "w_d_d")
                    b_v_sb = load_bias_bc(b_v[a], D, "b_row_d")
                    v_loc = act.tile([128, NT, D], DTM, tag="mix1", name="v_loc")

                    def into_v(t_, ps, n0, nw):
                        nc.vector.tensor_add(v_loc[:, t_, n0:n0 + nw], ps,
                                             b_v_sb[:, n0:n0 + nw])
                    mm_tm(xhatT, KC, w_v_sb, D, into_v)
                    # exchange V in two feature halves so heads 0-3 can
                    # start their PV matmuls after only the first collective
                    v_tm = big.tile([128, NKV, H, HD + 1], DTM, tag="v_tm", name="v_tm")
                    for vh in range(2):
                        c0 = vh * 256
                        v_bin = dram.tile([1024, 256], DTM, tag="v_bin", name="v_bin")
                        nc.sync.dma_start(v_bin[0:64], v_loc[64:128, 0, c0:c0 + 256])
                        nc.sync.dma_start(
                            v_bin[64:960].rearrange("(c p) n -> p c n", p=128),
                            v_loc[:, 1:8, c0:c0 + 256])
                        nc.sync.dma_start(v_bin[960:1024], v_loc[0:64, 8, c0:c0 + 256])
                        v_bout = dram.tile([S, 256], DTM, tag="v_bout", name="v_bout")
                        nc.gpsimd.collective_compute(
                            "AllGather", ALU.bypass, replica_groups=RG,
                            ins=[v_bin.opt()], outs=[v_bout.opt()])
                        for h_ in range(4):
                            nc.sync.dma_start(
                                v_tm[:, :, vh * 4 + h_, 0:HD],
                                v_bout[:, h_ * HD:(h_ + 1) * HD].rearrange(
                                    "(kc p) d -> p kc d", p=128))
                    nc.vector.memset(v_tm[:, :, :, HD:HD + 1], 1.0)

                    # Q projection (overlaps the collectives)
                    w_q_sb = load_w(w_q[a], KC, D, "w_d_d")
                    b_q_sb = load_bias_fm(b_q[a], KC, "b_fm_4")
                    q_fm = act.tile([128, KC, T], DTM, tag="q_fm", name="q_fm")
                    mm_fm(q_fm, xhatT, w_q_sb, b_q_sb, KC, AF.Identity)
                    w_qr_sb = load_w(w_qr[a], KC, D, "w_d_d")
                    b_qr_sb = load_bias_fm(b_qr[a], KC, "b_fm_4")
                    qr_fm = act.tile([128, KC, T], DTM, tag="rot", name="qr_fm")
                    mm_fm(qr_fm, xhatT, w_qr_sb, b_qr_sb, KC, AF.Identity)
                    rope_combine(q_fm, qr_fm)

                    # attention core: q restricted to the columns whose
                    # outputs are still needed (local + remaining conv halo)
                    ctx_fm = act.tile([128, KC, T], DTM, tag="mix1", name="ctx_fm")
                    nc.vector.memset(ctx_fm[:], 0.0)
                    for h in range(H):
                        r0 = (h % 2) * 64
                        ch = h // 2
                        av_tiles = [ps_av.tile([65, 512], F32, tag="ps_av", name="av")
                                    for _ in PANELS]
                        for c in range(NKV):
                            ex = sm3.tile([128, T], DTM, tag="ebuf", name="ex")
                            for j, (qo, qw) in enumerate(QPANELS):
                                ps = ps_mm.tile([128, 512], F32, tag="ps_mm", name="sc")
                                nc.tensor.matmul(
                                    ps[:, :qw],
                                    lhsT=k_full[r0:r0 + 64, ch, ts(c, 128)],
                                    rhs=q_fm[r0:r0 + 64, ch, qo:qo + qw],
                                    start=True, stop=True)
                                nc.scalar.activation(
                                    out=ex[:, qo:qo + qw], in_=ps[:, :qw],
                                    func=AF.Exp, scale=0.125)
                            for j, (qo, qw) in enumerate(QPANELS):
                                nc.tensor.matmul(
                                    av_tiles[j][:, :qw],
                                    lhsT=v_tm[:, c, h, :],
                                    rhs=ex[:, qo:qo + qw],
                                    start=(c == 0), stop=(c == NKV - 1))
                        inv = sm1.tile([1, T], F32, tag="attn_inv", name="inv")
                        invb = sm1.tile([64, T], F32, tag="attn_invb", name="invb")
                        for j, (qo, qw) in enumerate(QPANELS):
                            nc.vector.reciprocal(inv[:, qo:qo + qw],
                                                 av_tiles[j][64:65, :qw])
                            ib_ps = ps_tr.tile([64, 512], F32, tag="ps_tr", name="ib")
                            nc.tensor.matmul(ib_ps[:, :qw],
                                             lhsT=ones_f32[:],
                                             rhs=inv[:, qo:qo + qw],
                                             start=True, stop=True)
                            nc.vector.tensor_copy(invb[:, qo:qo + qw], ib_ps[:, :qw])
                        if r0 == 0:
                            for j, (qo, qw) in enumerate(QPANELS):
                                nc.vector.tensor_mul(
                                    ctx_fm[0:64, ch, qo:qo + qw],
                                    av_tiles[j][0:64, :qw], invb[:, qo:qo + qw])
                        else:
                            stage = sm2.tile([64, T], DTM, tag="ctx_stage", name="cst")
                            for j, (qo, qw) in enumerate(QPANELS):
                                nc.vector.tensor_mul(
                                    stage[:, qo:qo + qw],
                                    av_tiles[j][0:64, :qw], invb[:, qo:qo + qw])
                            nc.sync.dma_start(ctx_fm[64:128, ch, :], stage[:])

                    # output projection + residual
                    w_ao_sb = load_w(w_ao[a], KC, D, "w_d_d")
                    b_ao_sb = load_bias_bc(b_ao[a], D, "b_row_d")

                    def into_res(t_, ps, n0, nw):
                        nc.vector.tensor_add(x[:, t_, n0:n0 + nw],
                                             x[:, t_, n0:n0 + nw], ps)
                        nc.gpsimd.tensor_add(x[:, t_, n0:n0 + nw],
                                             x[:, t_, n0:n0 + nw],
                                             b_ao_sb[:, n0:n0 + nw])
                    mm_tm(ctx_fm, KC, w_ao_sb, D, into_res)

                    # === FF block (quarters over the 2048 hidden dim,
                    # each quarter's f2 partial sum added straight into x) ===
                    xhatT = layernorm_T(x)
                    for hf in range(4):
                        w_f1_sb = wpool.tile([128, KC, 512], DTM, tag="w_d_d",
                                             name="w_f1_sb")
                        nc.sync.dma_start(
                            w_f1_sb[:],
                            w_f1[a][:, hf * 512:(hf + 1) * 512].rearrange(
                                "(kc p) n -> p kc n", p=128))
                        b_f1_sb = wpool.tile([128, 4], F32, tag="b_fm_4", name="b_f1_sb")
                        nc.sync.dma_start(b_f1_sb[:], b_f1[a][:, hf * 4:(hf + 1) * 4])
                        g_fm = act.tile([128, 4, T], DTM, tag="mix1", name="g_fm")
                        mm_fm(g_fm, xhatT, w_f1_sb, b_f1_sb, 4, AF.Gelu)
                        w_f2_sb = wpool.tile([128, 4, D], DTM, tag="w_d_d",
                                             name="w_f2_sb")
                        nc.sync.dma_start(
                            w_f2_sb[:],
                            w_f2[a][hf * 512:(hf + 1) * 512].rearrange(
                                "(kc p) n -> p kc n", p=128))
                        if hf == 0:
                            b_f2_bc = load_bias_bc(b_f2[a], D, "b_row_d")

                            def into_ffres(t_, ps, n0, nw):
                                nc.vector.tensor_add(x[:, t_, n0:n0 + nw],
                                                     x[:, t_, n0:n0 + nw], ps)
                                nc.gpsimd.tensor_add(x[:, t_, n0:n0 + nw],
                                                     x[:, t_, n0:n0 + nw],
                                                     b_f2_bc[:, n0:n0 + nw])
                        else:
                            def into_ffres(t_, ps, n0, nw):
                                nc.vector.tensor_add(x[:, t_, n0:n0 + nw],
                                                     x[:, t_, n0:n0 + nw], ps)
                        mm_tm(g_fm, KC, w_f2_sb, D, into_ffres)
                else:
                    i = si; si += 1
                    # === SSM block ===
                    xhatT = layernorm_T(x)
                    w_si_sb = load_w(w_si[i], KC, 2 * D, "w_d_d2")
                    b_si_sb = load_bias_fm(b_si[i], 8, "b_fm_8")
                    z_fm = act.tile([128, 8, T], DTM, tag="g_fm", name="z_fm")
                    mm_fm(z_fm, xhatT, w_si_sb, b_si_sb, 4, AF.Sigmoid, off=0)
                    # gating + seq mask; cand buffer has a zero column each side
                    cand = act.tile([128, KC, T + 2], DTM, tag="mix2", name="cand")
                    nc.vector.memset(cand[:, :, 0:1], 0.0)
                    nc.vector.memset(cand[:, :, T + 1:T + 2], 0.0)
                    cw_sb = wpool.tile([128, KC, 3], F32, tag="cw", name="cw_sb")
                    nc.sync.dma_start(cw_sb[:], cw[i])
                    cb_sb = wpool.tile([128, KC], F32, tag="cb", name="cb_sb")
                    nc.sync.dma_start(cb_sb[:], cb[i])
                    conv_fm = act.tile([128, KC, T], DTM, tag="mix1", name="conv_fm")
                    # cand-chunk projection with gating fused into the PSUM
                    # epilogue: cand = ((z_cand + bias) * sigmoid(z_gate)) * mask
                    for n_ in range(KC):
                        for (qo, qw) in PANELS:
                            ps = ps_mm.tile([128, 512], F32, tag="ps_mm")
                            for kc_ in range(KC):
                                nc.tensor.matmul(
                                    ps[:, :qw], lhsT=w_si_sb[:, kc_, ts(KC + n_, 128)],
                                    rhs=xhatT[:, kc_, qo:qo + qw],
                                    start=(kc_ == 0), stop=(kc_ == KC - 1))
                            nc.vector.scalar_tensor_tensor(
                                out=cand[:, n_, 1 + qo:1 + qo + qw], in0=ps[:, :qw],
                                scalar=b_si_sb[:, KC + n_:KC + n_ + 1],
                                in1=z_fm[:, n_, qo:qo + qw],
                                op0=ALU.add, op1=ALU.mult)
                            nc.gpsimd.tensor_mul(cand[:, n_, 1 + qo:1 + qo + qw],
                                                 cand[:, n_, 1 + qo:1 + qo + qw],
                                                 smask_bc[:, qo:qo + qw])
                    for (o0, o1) in ((0, 640), (640, T)):
                        wd = o1 - o0
                        for c in range(KC):
                            t1 = sm2.tile([128, T], DTM, tag="ctmp_a", name="conv_t1")
                            t2 = sm2.tile([128, T], DTM, tag="ctmp_b", name="conv_t2")
                            nc.vector.tensor_scalar_mul(t1[:, :wd],
                                                        cand[:, c, o0:o1],
                                                        cw_sb[:, c, 0:1])
                            nc.vector.scalar_tensor_tensor(
                                out=t2[:, :wd], in0=cand[:, c, o0 + 1:o1 + 1],
                                scalar=cw_sb[:, c, 1:2], in1=t1[:, :wd],
                                op0=ALU.mult, op1=ALU.add)
                            nc.vector.scalar_tensor_tensor(
                                out=t1[:, :wd], in0=cand[:, c, o0 + 2:o1 + 2],
                                scalar=cw_sb[:, c, 2:3], in1=t2[:, :wd],
                                op0=ALU.mult, op1=ALU.add)
                            nc.scalar.activation(
                                out=conv_fm[:, c, o0:o1], in_=t1[:, :wd],
                                func=AF.Relu, bias=cb_sb[:, c:c + 1])
                    w_so_sb = load_w(w_so[i], KC, D, "w_d_d")
                    b_so_sb = load_bias_bc(b_so[i], D, "b_row_d")

                    def into_res_s(t_, ps, n0, nw):
                        nc.vector.tensor_add(x[:, t_, n0:n0 + nw],
                                             x[:, t_, n0:n0 + nw], ps)
                        nc.gpsimd.tensor_add(x[:, t_, n0:n0 + nw],
                                             x[:, t_, n0:n0 + nw],
                                             b_so_sb[:, n0:n0 + nw])
                    mm_tm(conv_fm, KC, w_so_sb, D, into_res_s)

            # ---------- heads ----------
            hT = layernorm_T(x)
            w_fr_sb = load_w(w_fr[:], KC, F, "w_d_d")
            b_fr_sb = load_bias_bc(b_fr[:], F, "b_row_d")

            def into_frame(t_, ps, n0, nw):
                st_ = sm2.tile([128, 512], F32, tag="ln_xh", name="ost")
                nc.vector.tensor_add(st_[:, :nw], ps, b_fr_sb[:, n0:n0 + nw])
                nc.sync.dma_start(out_frame[ts(t_, 128), n0:n0 + nw], st_[:, :nw])
            mm_tm(hT, KC, w_fr_sb, F, into_frame)

            w_sy_sb = wpool1.tile([128, KC, V], DTM, tag="w_d_d2", name="w_sy_sb")
            nc.sync.dma_start(w_sy_sb[:],
                              w_sy[:].rearrange("(kc p) n -> p kc n", p=128))
            b_sy_sb = wpool1.tile([128, V], DTM, tag="b_row_2d", name="b_sy_sb")
            nc.sync.dma_start(b_sy_sb[:], b_sy[:].to_broadcast((128, V)))

            def into_sym(t_, ps, n0, nw):
                st_ = sm2.tile([128, 512], F32, tag="ln_xh", name="ost")
                nc.vector.tensor_add(st_[:, :nw], ps, b_sy_sb[:, n0:n0 + nw])
                nc.sync.dma_start(out_sym[ts(t_, 128), n0:n0 + nw], st_[:, :nw])
            mm_tm(hT, KC, w_sy_sb, V, into_sym)

    nc.finalize()
    return nc


# ------------------------------------------------------------- host prep

def _host_prep(inputs, np_mm):
    g = {k: np.asarray(v, np.float32) if np.asarray(v).dtype != np.bool_
         else np.asarray(v) for k, v in inputs.items()}

    def fm_bias(b):      # [N] -> [128, N//128]
        return np.ascontiguousarray(b.reshape(-1, 128).T).astype(np.float32)

    P = {}
    P["w_in"] = g["input_w"].astype(np_mm)
    P["b_in"] = g["input_b"][None, :].astype(np_mm)
    wsi = np.empty((NSSM, D, 2 * D), np_mm); bsi = np.empty((NSSM, 128, 8), np.float32)
    cwl = np.empty((NSSM, 128, KC, 3), np.float32)
    cbl = np.empty((NSSM, 128, KC), np.float32)
    wso = np.empty((NSSM, D, D), np_mm); bso = np.empty((NSSM, 1, D), np_mm)
    for i in range(NSSM):
        nw, nb = g["ssm_norm_w"][i], g["ssm_norm_b"][i]
        wsi[i] = (nw[:, None] * g["ssm_in_w"][i]).astype(np_mm)
        bsi[i] = fm_bias(nb @ g["ssm_in_w"][i] + g["ssm_in_b"][i])
        cwl[i] = g["ssm_conv_w"][i].reshape(KC, 128, 3).transpose(1, 0, 2)
        cbl[i] = g["ssm_conv_b"][i].reshape(KC, 128).T
        wso[i] = g["ssm_out_w"][i].astype(np_mm)
        bso[i] = g["ssm_out_b"][i][None, :].astype(np_mm)
    P.update(w_si=wsi, b_si=bsi, cw=cwl, cb=cbl, w_so=wso, b_so=bso)

    def rot_feat(w):
        """Apply rotate_half permutation (with sign) to output features of w [.., D]."""
        wh = w.reshape(w.shape[:-1] + (H, HD))
        out = np.empty_like(wh)
        out[..., 0:32] = -wh[..., 32:64]
        out[..., 32:64] = wh[..., 0:32]
        return out.reshape(w.shape)

    wq = np.empty((NATTN, D, D), np_mm); bq = np.empty((NATTN, 128, KC), np.float32)
    wk = np.empty((NATTN, D, D), np_mm); bk = np.empty((NATTN, 128, KC), np.float32)
    wqr = np.empty((NATTN, D, D), np_mm); bqr = np.empty((NATTN, 128, KC), np.float32)
    wkr = np.empty((NATTN, D, D), np_mm); bkr = np.empty((NATTN, 128, KC), np.float32)
    wv = np.empty((NATTN, D, D), np_mm); bv = np.empty((NATTN, 1, D), np_mm)
    wao = np.empty((NATTN, D, D), np_mm); bao = np.empty((NATTN, 1, D), np_mm)
    wf1 = np.empty((NATTN, D, FFD), np_mm)
    bf1 = np.empty((NATTN, 128, FFD // 128), np.float32)
    wf2 = np.empty((NATTN, FFD, D), np_mm); bf2 = np.empty((NATTN, 1, D), np_mm)
    for a in range(NATTN):
        n1w, n1b = g["a_ln1_w"][a], g["a_ln1_b"][a]
        wqf = n1w[:, None] * g["a_q_w"][a]
        bqf = n1b @ g["a_q_w"][a] + g["a_q_b"][a]
        wkf = n1w[:, None] * g["a_k_w"][a]
        bkf = n1b @ g["a_k_w"][a] + g["a_k_b"][a]
        wq[a] = wqf.astype(np_mm); bq[a] = fm_bias(bqf)
        wk[a] = wkf.astype(np_mm); bk[a] = fm_bias(bkf)
        wqr[a] = rot_feat(wqf).astype(np_mm); bqr[a] = fm_bias(rot_feat(bqf))
        wkr[a] = rot_feat(wkf).astype(np_mm); bkr[a] = fm_bias(rot_feat(bkf))
        wv[a] = (n1w[:, None] * g["a_v_w"][a]).astype(np_mm)
        bv[a] = (n1b @ g["a_v_w"][a] + g["a_v_b"][a])[None, :].astype(np_mm)
        wao[a] = g["a_o_w"][a].astype(np_mm)
        bao[a] = g["a_o_b"][a][None, :].astype(np_mm)
        n2w, n2b = g["a_ln2_w"][a], g["a_ln2_b"][a]
        wf1[a] = (n2w[:, None] * g["a_f1_w"][a]).astype(np_mm)
        bf1[a] = fm_bias(n2b @ g["a_f1_w"][a] + g["a_f1_b"][a])
        wf2[a] = g["a_f2_w"][a].astype(np_mm)
        bf2[a] = g["a_f2_b"][a][None, :].astype(np_mm)
    P.update(w_q=wq, b_q=bq, w_k=wk, b_k=bk, w_qr=wqr, b_qr=bqr,
             w_kr=wkr, b_kr=bkr, w_v=wv, b_v=bv, w_ao=wao, b_ao=bao,
             w_f1=wf1, b_f1=bf1, w_f2=wf2, b_f2=bf2)

    fnw, fnb = g["fn_w"], g["fn_b"]
    P["w_fr"] = (fnw[:, None] * g["frame_w"]).astype(np_mm)
    P["b_fr"] = (fnb @ g["frame_w"] + g["frame_b"])[None, :].astype(np_mm)
    P["w_sy"] = (fnw[:, None] * g["sym_w"]).astype(np_mm)
    P["b_sy"] = (fnb @ g["sym_w"] + g["sym_b"])[None, :].astype(np_mm)

    frames = g["frames"]
    inv_freq = 1.0 / (10000.0 ** (np.arange(0, HD, 2, dtype=np.float32) / HD))
    per_core = []
    for c in range(8):
        b, hhalf = c // 2, c % 2
        start = hhalf * 1024 - HALO
        idx = np.arange(start, start + T)
        inseq = (idx >= 0) & (idx < S)
        fr = np.zeros((T, F), np.float32)
        fr[inseq] = frames[b][idx[inseq]]
        freqs = np.outer(idx.astype(np.float32), inv_freq)
        cos32 = np.cos(freqs).astype(np.float32)   # [T, 32]
        sin32 = np.sin(freqs).astype(np.float32)
        pr = np.arange(128) % 32
        d = dict(P)
        d["framesT"] = np.ascontiguousarray(fr.T).astype(np_mm)
        d["smask"] = inseq.astype(np.float32)[None, :].astype(np_mm)
        d["cosB"] = np.ascontiguousarray(cos32[:, pr].T).astype(np_mm)
        d["sinB"] = np.ascontiguousarray(sin32[:, pr].T).astype(np_mm)
        per_core.append(d)
    return per_core


# ----------------------------------------------------------------- entry

def kernel(**inputs):
    dt_mm = os.environ.get("MJM_DT", "bfloat16")
    if dt_mm == "bfloat16":
        import ml_dtypes
        np_mm = ml_dtypes.bfloat16
    else:
        np_mm = np.float32

    trace = os.environ.get("MJM_TRACE", "0") == "1"
    if trace:
        import sys, types
        if "antenv.axon_hooks" not in sys.modules:
            try:
                from trn_agent_boot.trn_boot import _ntff_profile_via_ctypes
                hook = _ntff_profile_via_ctypes("/opt/axon/libaxon_pjrt.so")
                mod = types.ModuleType("antenv.axon_hooks")
                mod.get_axon_ntff_profile_hook = lambda: hook
                sys.modules["antenv.axon_hooks"] = mod
            except Exception:
                trace = False

    from concourse.bass_utils import run_bass_kernel_spmd

    key = ("nc", dt_mm)
    if key not in _CACHE:
        _CACHE[key] = _build_nc(dt_mm)
    nc = _CACHE[key]

    in_maps = _host_prep(inputs, np_mm)
    last_err = None
    res = None
    for _attempt in range(3):
        try:
            res = run_bass_kernel_spmd(nc, in_maps, core_ids=list(range(8)),
                                       trace=trace)
            break
        except Exception as e:                       # transient NRT/worker errors
            last_err = e
            import time as _time
            _time.sleep(5)
    if res is None:
        raise last_err

    if trace and res.exec_time_ns is not None:
        print(f"HW exec time: {res.exec_time_ns} ns")

    frame_out = np.zeros((B, S, F), np.float32)
    sym_out = np.zeros((B, S, V), np.float32)
    for c in range(8):
        b, hhalf = c // 2, c % 2
        sl = slice(hhalf * 1024, hhalf * 1024 + 1024)
        frame_out[b, sl] = res.results[c]["out_frame"][HALO:HALO + 1024]
        sym_out[b, sl] = res.results[c]["out_sym"][HALO:HALO + 1024]
    return frame_out, sym_out
